# revision 1
# baseline (speedup 1.0000x reference)
"""Causal multi-head attention block (B=2, S=2048, D=1024, H=16) on 8 TRN2 cores.

Sharding: core i handles batch b = i//4 and head group hg = i%4 (4 heads =
256 model dims). Each core computes its heads' attention and a partial
output projection; the host sums the 4 partials per batch and adds b_out.

Per-core device pipeline (bf16 matmuls, fp32 PSUM accumulation):
  1. QKV. Q^T,K^T land as [head_cols, tokens] (lhsT = W, rhs = x^T);
     V lands as [tokens, head_cols] (lhsT = x^T tiles, rhs = W_v) and is
     stored augmented with a ones column so the attention z-matmul also
     produces softmax row sums.
  2. Attention per head, flash-style in the S^T = K.Q^T orientation over
     the causal lower triangle only: S^T[k_tile, q_span] -> exp on ScalarE
     (scale=1/8; no max subtraction, logits are ~N(0,1)) -> P^T bf16 ->
     multiplicative 0/1 mask on the diagonal block -> z^T[d+1, q] +=
     V_aug^T @ P^T accumulated over k tiles in PSUM. Consecutive k tiles
     share one S region so each exp call covers up to 1024 columns.
  3. Normalize as soon as a q-quarter's last k tile lands: recip(rowsum),
     GPSIMD partition-broadcast, z * recip on VectorE -> bf16 zT. The V
     bias is folded into the output bias on the host (b_v @ w_out).
  4. Out-proj: y_partial[t, n] accumulated over the 256 local dims.

Program order is a 4-stage pipeline over 512-token quarters --
QKV(tg0), att(qg0), QKV(tg1), att(qg1), ... out-proj last -- so ScalarE
exp work overlaps PE QKV work and out-proj fills late PE gaps. Host
pre-packs all inputs into SBUF layouts (bf16) for contiguous DMA.
"""

import numpy as np
import ml_dtypes

import concourse.mybir as mybir
import concourse.tile as tile
from concourse import bacc
from concourse.bass_utils import run_bass_kernel_spmd

B = 2
S = 2048
DM = 1024
HD = 64
HLOC = 4                 # heads per core
CLOC = HLOC * HD         # local model dims (256)
MO = DM // 128           # 8 k-subtiles of the model dim
NKT = S // 128           # 16 key tiles


f32 = mybir.dt.float32
bf16 = mybir.dt.bfloat16
EXP = mybir.ActivationFunctionType.Exp

_CACHE = {}


def build(ps_z_bufs=2, pt_bufs=5, op_engines=("dve", "dve", "dve", "mix"),
          interleave_heads=False, dma_splits=(1, 1, 1, 4), sp_bufs=4,
          yp_bufs=4, sreg_w=1024):
    nc = bacc.Bacc("TRN2", target_bir_lowering=False, debug=False)

    xT_d = nc.dram_tensor("xT", [128, MO, S], bf16, kind="ExternalInput")
    wqk_d = nc.dram_tensor("wqk", [128, MO, 2 * CLOC], bf16, kind="ExternalInput")
    wv_d = nc.dram_tensor("wv", [128, MO, CLOC], bf16, kind="ExternalInput")
    wo_d = nc.dram_tensor("wo", [128, 2, DM], bf16, kind="ExternalInput")
    # consts packed as raw bf16 columns: tri[0:128], bq[128:132],
    # bk[132:136], bv[136:144] (f32 values bit-split across bf16 pairs)
    cst_d = nc.dram_tensor("cst", [128, 144], bf16, kind="ExternalInput")
    y_d = nc.dram_tensor("y", [S, DM], f32, kind="ExternalOutput")

    with tile.TileContext(nc) as tc:
        with (
            tc.tile_pool(name="consts", bufs=1) as consts,
            tc.tile_pool(name="acts", bufs=1) as apool,
            tc.tile_pool(name="pt", bufs=pt_bufs) as ppool,
            tc.tile_pool(name="norm", bufs=sp_bufs) as spool,
            tc.tile_pool(name="ycopy", bufs=yp_bufs) as ypool,
            # 8 PSUM banks: ps_s 2x[128,1024]=4 (QKV Q/K + attention S),
            # ps_z [128,512] z accumulators, ps_b rest (V / out-proj)
            tc.tile_pool(name="ps_s", bufs=2, space="PSUM") as ps_s,
            tc.tile_pool(name="ps_z", bufs=ps_z_bufs, space="PSUM") as ps_z,
            tc.tile_pool(name="ps_b", bufs=8 - 2 * (sreg_w // 512) - ps_z_bufs,
                         space="PSUM") as ps_b,
        ):
            csb = consts.tile([128, 144], bf16)
            wqk = consts.tile([128, MO, 2 * CLOC], bf16)
            wv = consts.tile([128, MO, CLOC], bf16)
            wo = consts.tile([128, 2, DM], bf16)
            xT = apool.tile([128, MO, S], bf16)

            # DMA order = consumption order. First chunks are fine-grained
            # so the first QKV matmul starts ~2.5us in; the rest are big
            # transfers to minimize per-DMA descriptor overhead.
            nc.sync.dma_start(wqk[:, 0:3, 0:256], wqk_d[:, 0:3, 0:256])
            nc.scalar.dma_start(xT[:, 0:3, 0:512], xT_d[:, 0:3, 0:512])
            nc.sync.dma_start(csb[:], cst_d[:])
            nc.sync.dma_start(wqk[:, 3:MO, 0:256], wqk_d[:, 3:MO, 0:256])
            nc.scalar.dma_start(xT[:, 3:MO, 0:512], xT_d[:, 3:MO, 0:512])
            nc.sync.dma_start(wqk[:, :, 256:512], wqk_d[:, :, 256:512])
            nc.scalar.dma_start(wv[:], wv_d[:])
            nc.sync.dma_start(xT[:, :, 512:1024], xT_d[:, :, 512:1024])
            nc.scalar.dma_start(xT[:, :, 1024:1536], xT_d[:, :, 1024:1536])
            nc.sync.dma_start(xT[:, :, 1536:2048], xT_d[:, :, 1536:2048])
            nc.scalar.dma_start(wo[:], wo_d[:])

            tri = csb[:, 0:128]
            bq_sb = csb[:, 128:132].bitcast(f32)
            bk_sb = csb[:, 132:136].bitcast(f32)

            QT = apool.tile([128, 2, S], bf16)
            KT = apool.tile([128, 2, S], bf16)
            # V augmented: [t-part, kt, h, 0:64] = v dims, col 64 = ones
            VA = apool.tile([128, NKT, HLOC, 72], bf16)
            nc.vector.memset(VA[:, :, :, 64:65], 1.0)
            zT = apool.tile([128, 2, S], bf16)

            def emit_qkv_qk(tg, pool=None, ptag="s"):
                pool = pool or ps_s
                tsl = slice(tg * 512, (tg + 1) * 512)
                for ct in range(2):
                    for j, (dst, b_sb) in enumerate(
                        ((QT, bq_sb), (KT, bk_sb))
                    ):
                        csl = slice(ct * 256 + j * 128, ct * 256 + (j + 1) * 128)
                        ps = pool.tile([128, 512 if ptag == "b" else sreg_w],
                                       f32, tag=ptag,
                                       name=f"qk_{tg}_{ct}_{j}")
                        for mo in range(MO):
                            nc.tensor.matmul(
                                ps[:, 0:512],
                                wqk[:, mo, csl],
                                xT[:, mo, tsl],
                                start=(mo == 0),
                                stop=(mo == MO - 1),
                            )
                        nc.vector.tensor_scalar_add(
                            dst[:, ct, tsl], ps[:, 0:512], b_sb[:, ct : ct + 1]
                        )
            def emit_qkv_v(tg):
                for ti in range(4):
                    tt = tg * 4 + ti
                    ps = ps_b.tile([128, 512], f32, tag="b")
                    for mo in range(MO):
                        nc.tensor.matmul(
                            ps[:, 0:CLOC],
                            xT[:, mo, tt * 128 : (tt + 1) * 128],
                            wv[:, mo, :],
                            start=(mo == 0),
                            stop=(mo == MO - 1),
                        )
                    if tg == 0:
                        nc.scalar.copy(
                            VA[:, tt, :, 0:64],
                            ps[:, 0:CLOC].rearrange("p (h d) -> p h d", d=64),
                        )
                    else:
                        nc.vector.tensor_copy(
                            VA[:, tt, :, 0:64],
                            ps[:, 0:CLOC].rearrange("p (h d) -> p h d", d=64),
                        )

            def emit_attention(qg):
                g0 = qg * 512
                last_kt = 4 * qg + 3

                # pack consecutive k tiles into shared S regions so one
                # exp call covers up to 1024 columns
                groups, cur, cum = [], [], 0
                for kt in range(last_kt + 1):
                    w = g0 + 512 - max(kt * 128, g0)
                    if cum + w > sreg_w:
                        groups.append(cur)
                        cur, cum = [], 0
                    cur.append((kt, cum, w))
                    cum += w
                groups.append(cur)

                head_lists = ([0, 1, 3, 2] if not interleave_heads
                              else [[0, 1], [2, 3]])
                for hl in head_lists:
                    hs = [hl] if isinstance(hl, int) else hl
                    zp_map = {}
                    for h in hs:
                        zp_map[h] = ps_z.tile([128, 512], f32, tag="z",
                                              name=f"zps_{h}_{qg}")
                    for grp in groups:
                        for h in hs:
                            emit_head_grp(h, grp, zp_map[h], qg, g0, last_kt)
                    for h in hs:
                        emit_norm(h, zp_map[h], qg, g0)

            def emit_head_grp(h, grp, zp, qg, g0, last_kt):
                    hp = (h % 2) * 64
                    ct = h // 2
                    if True:
                        sreg = ps_s.tile([128, sreg_w], f32, tag="s",
                                         name=f"s_{h}_{qg}_{grp[0][0]}")
                        cum = grp[-1][1] + grp[-1][2]
                        for kt, off, w in grp:
                            q0 = g0 + 512 - w
                            c0 = off
                            while c0 < off + w:
                                cw = min(off + w - c0, 512 - c0 % 512)
                                nc.tensor.matmul(
                                    sreg[:, c0 : c0 + cw],
                                    KT[hp : hp + 64, ct,
                                       kt * 128 : (kt + 1) * 128],
                                    QT[hp : hp + 64, ct,
                                       q0 + c0 - off : q0 + c0 - off + cw],
                                )
                                c0 += cw
                        pT = ppool.tile([128, sreg_w], bf16, tag="pT")
                        nc.scalar.activation(
                            pT[:, :cum], sreg[:, :cum], EXP, scale=0.125
                        )
                        for kt, off, w in grp:
                            if kt * 128 >= g0:  # diagonal block leads span
                                nc.vector.tensor_mul(
                                    pT[:, off : off + 128],
                                    pT[:, off : off + 128],
                                    tri[:],
                                )
                            q0 = g0 + 512 - w
                            nc.tensor.matmul(
                                zp[0:65, q0 - g0 : 512],
                                VA[:, kt, h, 0:65],
                                pT[:, off : off + w],
                                start=(kt == 0),
                                stop=(kt == last_kt),
                            )

            def emit_norm(h, zp, qg, g0):
                    hp = (h % 2) * 64
                    ct = h // 2
                    rec32 = spool.tile([1, 512], f32, tag="rec32",
                                       name=f"rec_{h}_{qg}")
                    nc.vector.reciprocal(rec32[:], zp[64:65, 0:512])
                    bcast = spool.tile([64, 512], f32, tag="bcast",
                                       name=f"bc_{h}_{qg}")
                    nc.gpsimd.partition_broadcast(bcast[:], rec32[:])
                    # b_v is folded into b_out on the host:
                    # y += (1 (x) b_v) @ w_out is a constant row vector
                    with nc.allow_low_precision(reason="attn out to bf16"):
                        if hp == 0:
                            nc.vector.tensor_mul(
                                zT[0:64, ct, g0 : g0 + 512],
                                zp[0:64, 0:512], bcast[:],
                            )
                        else:
                            zbf = spool.tile([64, 512], bf16, tag="zbf",
                                             name=f"zb_{h}_{qg}")
                            nc.vector.tensor_mul(
                                zbf[:], zp[0:64, 0:512], bcast[:]
                            )
                            nc.sync.dma_start(
                                zT[hp : hp + 64, ct, g0 : g0 + 512], zbf[:]
                            )

            def emit_outproj(qg, copy_eng, dma_split=1):
                for nh in range(2):
                    ysb = ypool.tile([128, 4, 512], f32, tag="y",
                                     name=f"ysb_{qg}_{nh}")
                    nper = 4 // dma_split
                    for ti in range(4):
                        tt = qg * 4 + ti
                        ps = ps_b.tile([128, 512], f32, tag="b")
                        for co in range(2):
                            nc.tensor.matmul(
                                ps[:],
                                zT[:, co, tt * 128 : (tt + 1) * 128],
                                wo[:, co, nh * 512 : (nh + 1) * 512],
                                start=(co == 0),
                                stop=(co == 1),
                            )
                        eng = copy_eng if copy_eng != "mix" else (
                            "act" if (tt + nh) % 2 == 0 else "dve"
                        )
                        if eng == "act":
                            nc.scalar.copy(ysb[:, ti, :], ps[:])
                        else:
                            nc.vector.tensor_copy(ysb[:, ti, :], ps[:])
                        if ti % nper == nper - 1:
                            t0 = tt - nper + 1
                            deng = nc.sync if (ti // nper + nh) % 2 == 0 else nc.scalar
                            deng.dma_start(
                                y_d[t0 * 128 : (tt + 1) * 128,
                                    nh * 512 : (nh + 1) * 512].rearrange(
                                    "(ti p) n -> p ti n", p=128
                                ),
                                ysb[:, ti - nper + 1 : ti + 1, :],
                            )

            # 4-stage software pipeline: attention on quarter qg overlaps
            # the QKV projection of quarter qg+1 on PE
            emit_qkv_qk(0)
            emit_qkv_v(0)
            emit_attention(0)
            emit_qkv_qk(1)
            emit_qkv_v(1)
            emit_attention(1)
            emit_qkv_qk(2)
            emit_qkv_v(2)
            emit_attention(2)
            emit_qkv_qk(3)
            emit_qkv_v(3)
            emit_attention(3)
            for qg in range(4):
                emit_outproj(qg, op_engines[qg], dma_split=dma_splits[qg])

    nc.compile()
    return nc


def _pack_w(w):
    # [DM, C] -> [128, MO, C]: partition p holds rows {mo*128 + p}
    return np.ascontiguousarray(
        w.reshape(MO, 128, w.shape[1]).transpose(1, 0, 2)
    ).astype(ml_dtypes.bfloat16)


def make_in_maps(x, w_qkv, b_qkv, w_out):
    # multiplicative post-exp mask: 1 where k <= q (upper incl diag), else 0
    tri = np.tri(128, 128, 0, dtype=np.float32).T.astype(ml_dtypes.bfloat16)
    in_maps = []
    for core in range(8):
        b = core // 4
        hg = core % 4
        c0 = hg * CLOC
        csl = slice(c0, c0 + CLOC)

        # packed consts: [128, 144] bf16-typed raw columns
        cst = np.zeros((128, 144), np.uint16)
        cst[:, 0:128] = tri.view(np.uint16)
        bq = np.ascontiguousarray(
            b_qkv[csl].astype(np.float32).reshape(2, 128).T
        )
        bk = np.ascontiguousarray(
            b_qkv[DM + c0 : DM + c0 + CLOC].astype(np.float32).reshape(2, 128).T
        )
        bv = np.ascontiguousarray(
            b_qkv[2 * DM + c0 : 2 * DM + c0 + CLOC]
            .astype(np.float32).reshape(HLOC, 64).T
        )
        cst[:, 128:132] = bq.view(np.uint16).reshape(128, 4)
        cst[:, 132:136] = bk.view(np.uint16).reshape(128, 4)
        cst[0:64, 136:144] = bv.view(np.uint16).reshape(64, 8)

        wq_p = _pack_w(w_qkv[:, csl])
        wk_p = _pack_w(w_qkv[:, DM + c0 : DM + c0 + CLOC])
        wqk = np.concatenate(
            [wq_p[:, :, 0:128], wk_p[:, :, 0:128],
             wq_p[:, :, 128:256], wk_p[:, :, 128:256]],
            axis=2,
        )
        in_maps.append(
            {
                "xT": _pack_w(np.ascontiguousarray(x[b].T)),
                "wqk": np.ascontiguousarray(wqk),
                "wv": _pack_w(w_qkv[:, 2 * DM + c0 : 2 * DM + c0 + CLOC]),
                # wo: [CLOC, DM] -> [128, 2, DM]
                "wo": np.ascontiguousarray(
                    w_out[csl, :].reshape(2, 128, DM).transpose(1, 0, 2)
                ).astype(ml_dtypes.bfloat16),
                "cst": cst.view(ml_dtypes.bfloat16),
            }
        )
    return in_maps


def gather(results, b_qkv, w_out, b_out):
    # device skips the V bias; z_norm + b_v projects to a constant row:
    # y += b_v @ w_out, folded into the output bias here
    b_eff = (
        b_out.astype(np.float32)
        + b_qkv[2 * DM :].astype(np.float32) @ w_out.astype(np.float32)
    )
    out = np.empty((B, S, DM), np.float32)
    for b in range(B):
        acc = results[4 * b]["y"].astype(np.float32)
        for j in range(1, 4):
            acc = acc + results[4 * b + j]["y"]
        out[b] = acc + b_eff[None, :]
    return out


def kernel(x, w_qkv, b_qkv, w_out, b_out):
    x = np.asarray(x)
    w_qkv = np.asarray(w_qkv)
    b_qkv = np.asarray(b_qkv)
    w_out = np.asarray(w_out)
    b_out = np.asarray(b_out)

    if "nc" not in _CACHE:
        _CACHE["nc"] = build()
    nc = _CACHE["nc"]

    in_maps = make_in_maps(x, w_qkv, b_qkv, w_out)
    res = run_bass_kernel_spmd(nc, in_maps, core_ids=list(range(8)))
    return gather(res.results, b_qkv, w_out, b_out)



# revision 10
# speedup vs baseline: 1.0327x; 1.0327x over previous
"""Causal multi-head attention block (B=2, S=2048, D=1024, H=16) on 8 TRN2 cores.

Sharding: core i handles batch b = i//4 and head group hg = i%4 (4 heads =
256 model dims). Each core computes its heads' attention and a partial
output projection; the host sums the 4 partials per batch and adds b_out.

Per-core device pipeline (bf16 matmuls, fp32 PSUM accumulation):
  1. QKV. Q^T,K^T land as [head_cols, tokens] (lhsT = W, rhs = x^T);
     V lands as [tokens, head_cols] (lhsT = x^T tiles, rhs = W_v), stored
     augmented with a ones column so the z-matmul also produces softmax
     row sums.
  2. Attention per head, flash-style in the S^T = K.Q^T orientation over
     the causal lower triangle only: S^T[k_tile, q_span] -> exp on ScalarE
     (scale=1/8, no max subtraction; logits ~N(0,1)) -> P^T bf16 ->
     multiplicative 0/1 mask on diagonal blocks (GPSIMD) -> z[q_tile, 65]
     += P^T_chunk^T @ V_aug accumulated over k tiles in PSUM. The [q, d+1]
     z orientation makes each z matmul only 65 PE columns (vs a full
     q-span) and puts the softmax row sum in PSUM column 64 of the same
     partition as its query, so normalization is a per-partition
     tensor_scalar multiply fused into the PSUM->SBUF copy.
  3. z[q,d] tiles are transposed back to z^T[d,q] via PE transpose
     (identity matmul, 128 cols per 2-head tile) for the out-projection.
  4. Out-proj: y_partial[t, n] accumulated over the 256 local dims.

Program order is a fine-grained software pipeline: the attention loop is
a flat sequence over (q-quarter, head, k-group) with the z matmuls
lagging one group behind the S matmuls, and a filler queue (next token
group's QKV chains, previous quarters' out-proj chunks) drained between
S and z so the PE never waits on ScalarE exp. Host pre-packs all inputs
into SBUF layouts (bf16); the V bias is folded into the output bias on
the host (b_v @ w_out).
"""

import numpy as np
import ml_dtypes

import concourse.mybir as mybir
import concourse.tile as tile
from concourse import bacc
from concourse.bass_utils import run_bass_kernel_spmd

B = 2
S = 2048
DM = 1024
HD = 64
HLOC = 4                 # heads per core
CLOC = HLOC * HD         # local model dims (256)
MO = DM // 128           # 8 k-subtiles of the model dim
NKT = S // 128           # 16 key tiles

f32 = mybir.dt.float32
bf16 = mybir.dt.bfloat16
EXP = mybir.ActivationFunctionType.Exp

ACT_NS = 0.8333333333333334
PE_NS = 0.4166666666666667

_CACHE = {}


def _groups(qg, cap=1024):
    """Pack the causal k-tile spans of query quarter qg into exp groups of
    <= cap columns. Returns list of groups; each group is a list of
    (kt, offset_in_group, width)."""
    g0 = qg * 512
    last_kt = 4 * qg + 3
    groups, cur, cum = [], [], 0
    for kt in range(last_kt + 1):
        w = g0 + 512 - max(kt * 128, g0)
        if cum + w > cap:
            groups.append(cur)
            cur, cum = [], 0
        cur.append((kt, cum, w))
        cum += w
    groups.append(cur)
    return groups


def build(pt_bufs=16, zq_bufs=4, y_bufs=3, sreg_w=1024, fill_scale=1.0,
          fill_pad=150.0, dma_splits=(2, 2, 2, 4), tri_engine="gpsimd"):
    nc = bacc.Bacc("TRN2", target_bir_lowering=False, debug=False)

    xT_d = nc.dram_tensor("xT", [128, MO, S], bf16, kind="ExternalInput")
    wqk_d = nc.dram_tensor("wqk", [128, MO, 2 * CLOC], bf16, kind="ExternalInput")
    wv_d = nc.dram_tensor("wv", [128, MO, CLOC], bf16, kind="ExternalInput")
    wo_d = nc.dram_tensor("wo", [128, 2, DM], bf16, kind="ExternalInput")
    # consts packed as raw bf16 columns: tri[0:128], identity[128:256],
    # bq[256:260], bk[260:264] (f32 values bit-split across bf16 pairs)
    cst_d = nc.dram_tensor("cst", [128, 264], bf16, kind="ExternalInput")
    y_d = nc.dram_tensor("y", [S, DM], f32, kind="ExternalOutput")

    with tile.TileContext(nc) as tc:
        with (
            tc.tile_pool(name="consts", bufs=1) as consts,
            tc.tile_pool(name="acts", bufs=1) as apool,
            tc.tile_pool(name="pt", bufs=pt_bufs) as ppool,
            tc.tile_pool(name="zq", bufs=zq_bufs) as zqpool,
            tc.tile_pool(name="norm", bufs=4) as spool,
            tc.tile_pool(name="ycopy", bufs=y_bufs) as ypool,
            # 8 PSUM banks: ps_s 2x[128,1024]=4 (QK logits), ps_z
            # 2x[128,4,65]=2 (z accumulators), ps_b 2x[128,512]=2
            # (QKV / V / out-proj chains and z transposes)
            tc.tile_pool(name="ps_s", bufs=2, space="PSUM") as ps_s,
            tc.tile_pool(name="ps_z", bufs=2, space="PSUM") as ps_z,
            tc.tile_pool(name="ps_b", bufs=2, space="PSUM") as ps_b,
        ):
            csb = consts.tile([128, 264], bf16)
            wqk = consts.tile([128, MO, 2 * CLOC], bf16)
            wv = consts.tile([128, MO, CLOC], bf16)
            wo = consts.tile([128, 2, DM], bf16)
            xT = apool.tile([128, MO, S], bf16)

            # DMA order = consumption order. First chunks are fine-grained
            # so the first QKV matmul starts ~2.5us in; the rest are big
            # transfers to minimize per-DMA descriptor overhead.
            nc.sync.dma_start(wqk[:, 0:3, 0:256], wqk_d[:, 0:3, 0:256])
            nc.scalar.dma_start(xT[:, 0:3, 0:512], xT_d[:, 0:3, 0:512])
            nc.sync.dma_start(csb[:], cst_d[:])
            nc.sync.dma_start(wqk[:, 3:MO, 0:256], wqk_d[:, 3:MO, 0:256])
            nc.scalar.dma_start(xT[:, 3:MO, 0:512], xT_d[:, 3:MO, 0:512])
            nc.sync.dma_start(wqk[:, :, 256:512], wqk_d[:, :, 256:512])
            nc.scalar.dma_start(wv[:], wv_d[:])
            nc.sync.dma_start(xT[:, :, 512:1024], xT_d[:, :, 512:1024])
            nc.scalar.dma_start(xT[:, :, 1024:1536], xT_d[:, :, 1024:1536])
            nc.sync.dma_start(xT[:, :, 1536:2048], xT_d[:, :, 1536:2048])
            nc.scalar.dma_start(wo[:], wo_d[:])

            tri = csb[:, 0:128]
            ident = csb[:, 128:256]
            bq_sb = csb[:, 256:260].bitcast(f32)
            bk_sb = csb[:, 260:264].bitcast(f32)

            QT = apool.tile([128, 2, S], bf16)
            KT = apool.tile([128, 2, S], bf16)
            # V augmented: [t-part, kt, h, 0:64] = v dims, col 64 = ones
            VA = apool.tile([128, NKT, HLOC, 72], bf16)
            nc.vector.memset(VA[:, :, :, 64:65], 1.0)
            zT = apool.tile([128, 2, S], bf16)

            # ---- filler queue: PE work units drained while ScalarE exps ----
            fillers = []          # list of (key, pe_ns, thunk)
            fill_debt = [0.0]

            def fill(budget_ns):
                budget = budget_ns + fill_debt[0]
                spent = 0.0
                while fillers and spent < budget:
                    _, ns, thunk = fillers.pop(0)
                    thunk()
                    spent += ns
                fill_debt[0] = budget - spent if fillers else 0.0

            def drain(key):
                """Force-emit queued fillers matching key (dependency
                barrier: attention on quarter qg needs all of QKV(tg=qg))."""
                rest = []
                for k, ns, thunk in fillers:
                    if k == key:
                        thunk()
                    else:
                        rest.append((k, ns, thunk))
                fillers[:] = rest

            def emit_qk_chain(tg, ct, j):
                tsl = slice(tg * 512, (tg + 1) * 512)
                csl = slice(ct * 256 + j * 128, ct * 256 + (j + 1) * 128)
                dst, b_sb = ((QT, bq_sb), (KT, bk_sb))[j]
                ps = ps_b.tile([128, 512], f32, tag="b",
                               name=f"qk_{tg}_{ct}_{j}")
                for mo in range(MO):
                    nc.tensor.matmul(
                        ps[:],
                        wqk[:, mo, csl],
                        xT[:, mo, tsl],
                        start=(mo == 0),
                        stop=(mo == MO - 1),
                    )
                nc.vector.tensor_scalar_add(
                    dst[:, ct, tsl], ps[:], b_sb[:, ct : ct + 1]
                )

            def emit_v_chain(tg, ti):
                tt = tg * 4 + ti
                ps = ps_b.tile([128, 512], f32, tag="b", name=f"v_{tg}_{ti}")
                for mo in range(MO):
                    nc.tensor.matmul(
                        ps[:, 0:CLOC],
                        xT[:, mo, tt * 128 : (tt + 1) * 128],
                        wv[:, mo, :],
                        start=(mo == 0),
                        stop=(mo == MO - 1),
                    )
                nc.vector.tensor_copy(
                    VA[:, tt, :, 0:64],
                    ps[:, 0:CLOC].rearrange("p (h d) -> p h d", d=64),
                )

            def emit_qkv(tg):
                for ct in range(2):
                    for j in range(2):
                        emit_qk_chain(tg, ct, j)
                for ti in range(4):
                    emit_v_chain(tg, ti)

            def push_qkv_fillers(tg):
                for ct in range(2):
                    for j in range(2):
                        fillers.append(
                            (("qkv", tg), 4096 * PE_NS,
                             lambda tg=tg, ct=ct, j=j: emit_qk_chain(tg, ct, j))
                        )
                for ti in range(4):
                    fillers.append(
                        (("qkv", tg), 2048 * PE_NS,
                         lambda tg=tg, ti=ti: emit_v_chain(tg, ti))
                    )

            # ---- attention ----
            def emit_sgrp(h, qg, gi, grp):
                """S^T matmuls for one exp group + the exp + diag masks."""
                hp = (h % 2) * 64
                ct = h // 2
                g0 = qg * 512
                cum = grp[-1][1] + grp[-1][2]
                sreg = ps_s.tile([128, sreg_w], f32, tag="s",
                                 name=f"s_{h}_{qg}_{gi}")
                for kt, off, w in grp:
                    q0 = g0 + 512 - w
                    c0 = off
                    while c0 < off + w:
                        cw = min(off + w - c0, 512 - c0 % 512)
                        nc.tensor.matmul(
                            sreg[:, c0 : c0 + cw],
                            KT[hp : hp + 64, ct, kt * 128 : (kt + 1) * 128],
                            QT[hp : hp + 64, ct,
                               q0 + c0 - off : q0 + c0 - off + cw],
                        )
                        c0 += cw
                pT = ppool.tile([128, sreg_w], bf16, tag="pT",
                                name=f"pT_{h}_{qg}_{gi}")
                nc.scalar.activation(pT[:, :cum], sreg[:, :cum], EXP,
                                     scale=0.125)
                teng = nc.gpsimd if tri_engine == "gpsimd" else nc.vector
                for kt, off, w in grp:
                    if kt * 128 >= g0:  # diagonal block leads the span
                        teng.tensor_mul(
                            pT[:, off : off + 128],
                            pT[:, off : off + 128],
                            tri[:],
                        )
                return pT

            def emit_zchain(h, qg, qt, pts, kt2g, zp):
                """z[q,65] = sum_kt pT_chunk^T @ V_aug: one sequential PSUM
                accumulation chain per q-tile (a PSUM bank supports only one
                open accumulation group at a time)."""
                g0 = qg * 512
                qa = 4 * qg + qt
                for kt in range(qa + 1):
                    gi, off, w = kt2g[kt]
                    q0 = g0 + 512 - w
                    c0 = off + (g0 + qt * 128) - q0
                    nc.tensor.matmul(
                        zp[:, qt, 0:65],
                        pts[gi][:, c0 : c0 + 128],
                        VA[:, kt, h, 0:65],
                        start=(kt == 0),
                        stop=(kt == qa),
                    )

            def emit_norm(h, qg, zp, zq):
                """1/rowsum fused into the PSUM->SBUF copy of z."""
                hp = (h % 2) * 64
                rec = spool.tile([128, 4, 1], f32, tag="rec",
                                 name=f"rec_{h}_{qg}")
                nc.vector.reciprocal(rec[:], zp[:, :, 64:65])
                with nc.allow_low_precision(reason="attn out to bf16"):
                    for qt in range(4):
                        nc.vector.tensor_scalar_mul(
                            zq[:, qt, hp : hp + 64],
                            zp[:, qt, 0:64],
                            rec[:, qt, :],
                        )

            def emit_transpose(qg, pair, zq):
                """zq [q,128d] -> zT [128d, q] via PE transpose of 4 tiles."""
                quad = ps_b.tile([128, 4, 128], bf16, tag="b",
                                 name=f"tq_{qg}_{pair}")
                for qt in range(4):
                    nc.tensor.transpose(quad[:, qt, :], zq[:, qt, :], ident)
                with nc.allow_low_precision(reason="zT copy"):
                    nc.vector.tensor_copy(
                        zT[:, pair, qg * 512 : (qg + 1) * 512],
                        quad[:].rearrange("p a b -> p (a b)"),
                    )

            def emit_op_chunk(qg, nh, ti, ysb, dma_split):
                tt = qg * 4 + ti
                ps = ps_b.tile([128, 512], f32, tag="b",
                               name=f"op_{qg}_{nh}_{ti}")
                for co in range(2):
                    nc.tensor.matmul(
                        ps[:],
                        zT[:, co, tt * 128 : (tt + 1) * 128],
                        wo[:, co, nh * 512 : (nh + 1) * 512],
                        start=(co == 0),
                        stop=(co == 1),
                    )
                nc.vector.tensor_copy(ysb[:, ti, :], ps[:])
                nper = 4 // dma_split
                if ti % nper == nper - 1:
                    t0 = tt - nper + 1
                    deng = nc.sync if (ti // nper + nh) % 2 == 0 else nc.scalar
                    deng.dma_start(
                        y_d[t0 * 128 : (tt + 1) * 128,
                            nh * 512 : (nh + 1) * 512].rearrange(
                            "(ti p) n -> p ti n", p=128
                        ),
                        ysb[:, ti - nper + 1 : ti + 1, :],
                    )

            def push_op_fillers(qg):
                dma_split = dma_splits[qg]
                for nh in range(2):
                    ysb = ypool.tile([128, 4, 512], f32, tag="y",
                                     name=f"ysb_{qg}_{nh}")
                    for ti in range(4):
                        fillers.append(
                            (("op", qg), 1024 * PE_NS,
                             lambda qg=qg, nh=nh, ti=ti, ysb=ysb,
                                    ds=dma_split:
                                 emit_op_chunk(qg, nh, ti, ysb, ds))
                        )

            def push_z_phase(qg, h, pts, kt2g, zq_box):
                """Queue head h's z chains + normalize (+ transpose) at the
                FRONT of the filler queue; they drain during head h+1's S
                phase (one-head software pipeline)."""
                box = {}

                def chain(qt):
                    if qt == 0:
                        box["zp"] = ps_z.tile([128, 4, 65], f32, tag="z",
                                              name=f"zp_{h}_{qg}")
                        if h % 2 == 0:
                            zq_box[h // 2] = zqpool.tile(
                                [128, 4, 128], bf16, tag="zq",
                                name=f"zq_{qg}_{h // 2}")
                    emit_zchain(h, qg, qt, pts, kt2g, box["zp"])

                def norm():
                    emit_norm(h, qg, box["zp"], zq_box[h // 2])
                    if h % 2 == 1:
                        emit_transpose(qg, h // 2, zq_box[h // 2])
                        if h == HLOC - 1:
                            # quarter finished: queue its out-proj (reads
                            # zT(qg), complete as of this point) and the
                            # next token group's QKV
                            push_op_fillers(qg)
                            if qg + 2 <= 3:
                                push_qkv_fillers(qg + 2)

                thunks = []
                for qt in range(4):
                    ncols = (4 * qg + qt + 1) * 65
                    thunks.append(
                        (("z", qg, h), ncols * PE_NS,
                         lambda qt=qt: chain(qt))
                    )
                thunks.append((("z", qg, h), 0.0, norm))
                fillers[0:0] = thunks

            # ---- program ----
            emit_qkv(0)
            push_qkv_fillers(1)

            zq_box = {}
            for qg in range(4):
                groups = _groups(qg, sreg_w)
                kt2g = {}
                for gi, grp in enumerate(groups):
                    for kt, off, w in grp:
                        kt2g[kt] = (gi, off, w)
                if qg > 0:
                    # barrier: this quarter's S/z read QT/KT/VA of tg=qg
                    drain(("qkv", qg))
                for h in range(HLOC):
                    pts = []
                    for gi, grp in enumerate(groups):
                        pts.append(emit_sgrp(h, qg, gi, grp))
                        cum = grp[-1][1] + grp[-1][2]
                        exp_ns = cum * ACT_NS + 185.0
                        budget = (exp_ns * fill_scale + fill_pad
                                  - cum * PE_NS)
                        fill(max(0.0, budget))
                    push_z_phase(qg, h, pts, kt2g, zq_box)

            # drain the tail (queue can grow while draining)
            while fillers:
                _, _, thunk = fillers.pop(0)
                thunk()

    nc.compile()
    return nc


def _pack_w(w):
    # [DM, C] -> [128, MO, C]: partition p holds rows {mo*128 + p}
    return np.ascontiguousarray(
        w.reshape(MO, 128, w.shape[1]).transpose(1, 0, 2)
    ).astype(ml_dtypes.bfloat16)


def make_in_maps(x, w_qkv, b_qkv, w_out):
    # multiplicative post-exp mask: 1 where k <= q (upper incl diag), else 0
    tri = np.tri(128, 128, 0, dtype=np.float32).T.astype(ml_dtypes.bfloat16)
    ident = np.eye(128, dtype=np.float32).astype(ml_dtypes.bfloat16)
    in_maps = []
    for core in range(8):
        b = core // 4
        hg = core % 4
        c0 = hg * CLOC
        csl = slice(c0, c0 + CLOC)

        # packed consts: [128, 264] bf16-typed raw columns
        cst = np.zeros((128, 264), np.uint16)
        cst[:, 0:128] = tri.view(np.uint16)
        cst[:, 128:256] = ident.view(np.uint16)
        bq = np.ascontiguousarray(
            b_qkv[csl].astype(np.float32).reshape(2, 128).T
        )
        bk = np.ascontiguousarray(
            b_qkv[DM + c0 : DM + c0 + CLOC].astype(np.float32).reshape(2, 128).T
        )
        cst[:, 256:260] = bq.view(np.uint16).reshape(128, 4)
        cst[:, 260:264] = bk.view(np.uint16).reshape(128, 4)

        wq_p = _pack_w(w_qkv[:, csl])
        wk_p = _pack_w(w_qkv[:, DM + c0 : DM + c0 + CLOC])
        wqk = np.concatenate(
            [wq_p[:, :, 0:128], wk_p[:, :, 0:128],
             wq_p[:, :, 128:256], wk_p[:, :, 128:256]],
            axis=2,
        )
        in_maps.append(
            {
                "xT": _pack_w(np.ascontiguousarray(x[b].T)),
                "wqk": np.ascontiguousarray(wqk),
                "wv": _pack_w(w_qkv[:, 2 * DM + c0 : 2 * DM + c0 + CLOC]),
                # wo: [CLOC, DM] -> [128, 2, DM]
                "wo": np.ascontiguousarray(
                    w_out[csl, :].reshape(2, 128, DM).transpose(1, 0, 2)
                ).astype(ml_dtypes.bfloat16),
                "cst": cst.view(ml_dtypes.bfloat16),
            }
        )
    return in_maps


def gather(results, b_qkv, w_out, b_out):
    # device skips the V bias; z_norm + b_v projects to a constant row:
    # y += b_v @ w_out, folded into the output bias here
    b_eff = (
        b_out.astype(np.float32)
        + b_qkv[2 * DM :].astype(np.float32) @ w_out.astype(np.float32)
    )
    out = np.empty((B, S, DM), np.float32)
    for b in range(B):
        acc = results[4 * b]["y"].astype(np.float32)
        for j in range(1, 4):
            acc = acc + results[4 * b + j]["y"]
        out[b] = acc + b_eff[None, :]
    return out


def kernel(x, w_qkv, b_qkv, w_out, b_out):
    x = np.asarray(x)
    w_qkv = np.asarray(w_qkv)
    b_qkv = np.asarray(b_qkv)
    w_out = np.asarray(w_out)
    b_out = np.asarray(b_out)

    if "nc" not in _CACHE:
        _CACHE["nc"] = build()
    nc = _CACHE["nc"]

    in_maps = make_in_maps(x, w_qkv, b_qkv, w_out)
    res = run_bass_kernel_spmd(nc, in_maps, core_ids=list(range(8)))
    return gather(res.results, b_qkv, w_out, b_out)


# revision 18
# speedup vs baseline: 1.0386x; 1.0057x over previous
"""Causal multi-head attention block (B=2, S=2048, D=1024, H=16) on 8 TRN2 cores.

Sharding: core i handles batch b = i//4 and head group hg = i%4 (4 heads =
256 model dims). Each core computes its heads' attention and a partial
output projection; the host sums the 4 partials per batch and adds b_out.

Per-core device pipeline (bf16 matmuls, fp32 PSUM accumulation):
  1. QKV. Q^T,K^T land as [head_cols, tokens] (lhsT = W, rhs = x^T);
     V lands as [tokens, head_cols] (lhsT = x^T tiles, rhs = W_v), stored
     augmented with a ones column so the z-matmul also produces softmax
     row sums.
  2. Attention per head, flash-style in the S^T = K.Q^T orientation over
     the causal lower triangle only: S^T[k_tile, q_span] -> exp on ScalarE
     (scale=1/8, no max subtraction; logits ~N(0,1)) -> P^T bf16 ->
     multiplicative 0/1 mask on diagonal blocks (GPSIMD) -> z[q_tile, 65]
     += P^T_chunk^T @ V_aug accumulated over k tiles in PSUM. The [q, d+1]
     z orientation makes each z matmul only 65 PE columns (vs a full
     q-span) and puts the softmax row sum in PSUM column 64 of the same
     partition as its query, so normalization is a per-partition
     tensor_scalar multiply fused into the PSUM->SBUF copy.
  3. z[q,d] tiles are transposed back to z^T[d,q] via PE transpose
     (identity matmul, 128 cols per 2-head tile) for the out-projection.
  4. Out-proj: y_partial[t, n] accumulated over the 256 local dims.

Program order is a fine-grained software pipeline: the attention loop is
a flat sequence over (q-quarter, head, k-group) with the z matmuls
lagging one group behind the S matmuls, and a filler queue (next token
group's QKV chains, previous quarters' out-proj chunks) drained between
S and z so the PE never waits on ScalarE exp. Host pre-packs all inputs
into SBUF layouts (bf16); the V bias is folded into the output bias on
the host (b_v @ w_out).
"""

import numpy as np
import ml_dtypes

import concourse.mybir as mybir
import concourse.tile as tile
from concourse import bacc
from concourse.bass_utils import run_bass_kernel_spmd

B = 2
S = 2048
DM = 1024
HD = 64
HLOC = 4                 # heads per core
CLOC = HLOC * HD         # local model dims (256)
MO = DM // 128           # 8 k-subtiles of the model dim
NKT = S // 128           # 16 key tiles

f32 = mybir.dt.float32
bf16 = mybir.dt.bfloat16
EXP = mybir.ActivationFunctionType.Exp

ACT_NS = 0.8333333333333334
PE_NS = 0.4166666666666667

_CACHE = {}


def _groups(qg, cap=1024):
    """Pack the causal k-tile spans of query quarter qg into exp groups of
    <= cap columns. Returns list of groups; each group is a list of
    (kt, offset_in_group, width)."""
    g0 = qg * 512
    last_kt = 4 * qg + 3
    groups, cur, cum = [], [], 0
    for kt in range(last_kt + 1):
        w = g0 + 512 - max(kt * 128, g0)
        if cum + w > cap:
            groups.append(cur)
            cur, cum = [], 0
        cur.append((kt, cum, w))
        cum += w
    groups.append(cur)
    return groups


def build(pt_bufs=16, zq_bufs=4, y_bufs=3, sreg_w=1024, fill_scale=1.0,
          fill_pad=150.0, dma_splits=(2, 2, 2, 4), tri_engine="gpsimd"):
    nc = bacc.Bacc("TRN2", target_bir_lowering=False, debug=False)

    xT_d = nc.dram_tensor("xT", [128, MO, S], bf16, kind="ExternalInput")
    # wqk grouped per QKV chain (ctj = ct*2+j) so each chain's weights are
    # one contiguous 2KB/partition DMA
    wqk_d = nc.dram_tensor("wqk", [128, 4, MO, 128], bf16, kind="ExternalInput")
    wv_d = nc.dram_tensor("wv", [128, MO, CLOC], bf16, kind="ExternalInput")
    wo_d = nc.dram_tensor("wo", [128, 2, DM], bf16, kind="ExternalInput")
    # consts packed as raw bf16 columns: tri[0:128], identity[128:256],
    # bq[256:260], bk[260:264] (f32 values bit-split across bf16 pairs)
    cst_d = nc.dram_tensor("cst", [128, 264], bf16, kind="ExternalInput")
    y_d = nc.dram_tensor("y", [S, DM], f32, kind="ExternalOutput")

    with tile.TileContext(nc) as tc:
        with (
            tc.tile_pool(name="consts", bufs=1) as consts,
            tc.tile_pool(name="acts", bufs=1) as apool,
            tc.tile_pool(name="pt", bufs=pt_bufs) as ppool,
            tc.tile_pool(name="zq", bufs=zq_bufs) as zqpool,
            tc.tile_pool(name="norm", bufs=4) as spool,
            tc.tile_pool(name="ycopy", bufs=y_bufs) as ypool,
            # 8 PSUM banks: ps_s 2x[128,1024]=4 (QK logits), ps_z
            # 2x[128,4,65]=2 (z accumulators), ps_b 2x[128,512]=2
            # (QKV / V / out-proj chains and z transposes)
            tc.tile_pool(name="ps_s", bufs=2, space="PSUM") as ps_s,
            tc.tile_pool(name="ps_z", bufs=2, space="PSUM") as ps_z,
            tc.tile_pool(name="ps_b", bufs=2, space="PSUM") as ps_b,
        ):
            csb = consts.tile([128, 264], bf16)
            wqk = consts.tile([128, 4, MO, 128], bf16)
            wv = consts.tile([128, MO, CLOC], bf16)
            wo = consts.tile([128, 2, DM], bf16)
            xT = apool.tile([128, MO, S], bf16)

            # DMA order = consumption order (transfers serialize on the DMA
            # engines): first QKV chain's weights, then xT mo-pairs at the
            # pace the first chain consumes them, then the rest in bulk.
            nc.sync.dma_start(wqk[:, 0, :, :], wqk_d[:, 0, :, :])
            nc.scalar.dma_start(xT[:, 0:2, 0:512], xT_d[:, 0:2, 0:512])
            nc.sync.dma_start(csb[:], cst_d[:])
            nc.scalar.dma_start(xT[:, 2:4, 0:512], xT_d[:, 2:4, 0:512])
            nc.sync.dma_start(wqk[:, 1, :, :], wqk_d[:, 1, :, :])
            nc.scalar.dma_start(xT[:, 4:6, 0:512], xT_d[:, 4:6, 0:512])
            nc.sync.dma_start(wqk[:, 2:4, :, :], wqk_d[:, 2:4, :, :])
            nc.scalar.dma_start(xT[:, 6:8, 0:512], xT_d[:, 6:8, 0:512])
            nc.sync.dma_start(wv[:], wv_d[:])
            nc.scalar.dma_start(xT[:, :, 512:1024], xT_d[:, :, 512:1024])
            nc.sync.dma_start(xT[:, :, 1024:1536], xT_d[:, :, 1024:1536])
            nc.scalar.dma_start(xT[:, :, 1536:2048], xT_d[:, :, 1536:2048])
            nc.sync.dma_start(wo[:], wo_d[:])

            tri = csb[:, 0:128]
            ident = csb[:, 128:256]
            bq_sb = csb[:, 256:260].bitcast(f32)
            bk_sb = csb[:, 260:264].bitcast(f32)

            QT = apool.tile([128, 2, S], bf16)
            KT = apool.tile([128, 2, S], bf16)
            # V augmented: [t-part, kt, h, 0:64] = v dims, col 64 = ones
            VA = apool.tile([128, NKT, HLOC, 72], bf16)
            nc.vector.memset(VA[:, :, :, 64:65], 1.0)
            zT = apool.tile([128, 2, S], bf16)

            # ---- filler queue: PE work units drained while ScalarE exps ----
            fillers = []          # list of (key, pe_ns, thunk)
            fill_debt = [0.0]

            def _pop_next():
                """z thunks first (pipeline steady state), then QKV (needed
                by the next quarter anyway), out-proj last (the only filler
                class available during the final ACT-bound quarter)."""
                for cls in ("z", "qkv"):
                    for i, (k, ns, thunk) in enumerate(fillers):
                        if k[0] == cls:
                            return fillers.pop(i)
                return fillers.pop(0)

            def fill(budget_ns):
                budget = budget_ns + fill_debt[0]
                spent = 0.0
                while fillers and spent < budget:
                    _, ns, thunk = _pop_next()
                    thunk()
                    spent += ns
                fill_debt[0] = budget - spent if fillers else 0.0

            def drain(key):
                """Force-emit queued fillers matching key (dependency
                barrier: attention on quarter qg needs all of QKV(tg=qg))."""
                rest = []
                for k, ns, thunk in fillers:
                    if k == key:
                        thunk()
                    else:
                        rest.append((k, ns, thunk))
                fillers[:] = rest

            def drain_class(cls):
                rest = []
                for k, ns, thunk in fillers:
                    if k[0] == cls:
                        thunk()
                    else:
                        rest.append((k, ns, thunk))
                fillers[:] = rest

            def emit_qk_chain(tg, ct, j):
                tsl = slice(tg * 512, (tg + 1) * 512)
                dst, b_sb = ((QT, bq_sb), (KT, bk_sb))[j]
                ps = ps_b.tile([128, 512], f32, tag="b",
                               name=f"qk_{tg}_{ct}_{j}")
                for mo in range(MO):
                    nc.tensor.matmul(
                        ps[:],
                        wqk[:, ct * 2 + j, mo, :],
                        xT[:, mo, tsl],
                        start=(mo == 0),
                        stop=(mo == MO - 1),
                    )
                nc.vector.tensor_scalar_add(
                    dst[:, ct, tsl], ps[:], b_sb[:, ct : ct + 1]
                )

            def emit_v_chain(tg, ti):
                tt = tg * 4 + ti
                ps = ps_b.tile([128, 512], f32, tag="b", name=f"v_{tg}_{ti}")
                for mo in range(MO):
                    nc.tensor.matmul(
                        ps[:, 0:CLOC],
                        xT[:, mo, tt * 128 : (tt + 1) * 128],
                        wv[:, mo, :],
                        start=(mo == 0),
                        stop=(mo == MO - 1),
                    )
                nc.vector.tensor_copy(
                    VA[:, tt, :, 0:64],
                    ps[:, 0:CLOC].rearrange("p (h d) -> p h d", d=64),
                )

            def emit_qkv(tg):
                for ct in range(2):
                    for j in range(2):
                        emit_qk_chain(tg, ct, j)
                for ti in range(4):
                    emit_v_chain(tg, ti)

            def push_qkv_fillers(tg):
                for ct in range(2):
                    for j in range(2):
                        fillers.append(
                            (("qkv", tg), 4096 * PE_NS,
                             lambda tg=tg, ct=ct, j=j: emit_qk_chain(tg, ct, j))
                        )
                for ti in range(4):
                    fillers.append(
                        (("qkv", tg), 2048 * PE_NS,
                         lambda tg=tg, ti=ti: emit_v_chain(tg, ti))
                    )

            # ---- attention ----
            def emit_sgrp(h, qg, gi, grp):
                """S^T matmuls for one exp group + the exp + diag masks."""
                hp = (h % 2) * 64
                ct = h // 2
                g0 = qg * 512
                cum = grp[-1][1] + grp[-1][2]
                sreg = ps_s.tile([128, sreg_w], f32, tag="s",
                                 name=f"s_{h}_{qg}_{gi}")
                for kt, off, w in grp:
                    q0 = g0 + 512 - w
                    c0 = off
                    while c0 < off + w:
                        cw = min(off + w - c0, 512 - c0 % 512)
                        nc.tensor.matmul(
                            sreg[:, c0 : c0 + cw],
                            KT[hp : hp + 64, ct, kt * 128 : (kt + 1) * 128],
                            QT[hp : hp + 64, ct,
                               q0 + c0 - off : q0 + c0 - off + cw],
                        )
                        c0 += cw
                pT = ppool.tile([128, sreg_w], bf16, tag="pT",
                                name=f"pT_{h}_{qg}_{gi}")
                nc.scalar.activation(pT[:, :cum], sreg[:, :cum], EXP,
                                     scale=0.125)
                teng = nc.gpsimd if tri_engine == "gpsimd" else nc.vector
                for kt, off, w in grp:
                    if kt * 128 >= g0:  # diagonal block leads the span
                        teng.tensor_mul(
                            pT[:, off : off + 128],
                            pT[:, off : off + 128],
                            tri[:],
                        )
                return pT

            def emit_zchain(h, qg, qt, pts, kt2g, zp):
                """z[q,65] = sum_kt pT_chunk^T @ V_aug: one sequential PSUM
                accumulation chain per q-tile (a PSUM bank supports only one
                open accumulation group at a time)."""
                g0 = qg * 512
                qa = 4 * qg + qt
                for kt in range(qa + 1):
                    gi, off, w = kt2g[kt]
                    q0 = g0 + 512 - w
                    c0 = off + (g0 + qt * 128) - q0
                    nc.tensor.matmul(
                        zp[:, qt, 0:65],
                        pts[gi][:, c0 : c0 + 128],
                        VA[:, kt, h, 0:65],
                        start=(kt == 0),
                        stop=(kt == qa),
                    )

            def emit_norm(h, qg, zp, zq):
                """1/rowsum fused into the PSUM->SBUF copy of z."""
                hp = (h % 2) * 64
                rec = spool.tile([128, 4, 1], f32, tag="rec",
                                 name=f"rec_{h}_{qg}")
                nc.vector.reciprocal(rec[:], zp[:, :, 64:65])
                with nc.allow_low_precision(reason="attn out to bf16"):
                    for qt in range(4):
                        nc.vector.tensor_scalar_mul(
                            zq[:, qt, hp : hp + 64],
                            zp[:, qt, 0:64],
                            rec[:, qt, :],
                        )

            def emit_transpose(qg, pair, zq):
                """zq [q,128d] -> zT [128d, q] via PE transpose of 4 tiles."""
                quad = ps_b.tile([128, 4, 128], bf16, tag="b",
                                 name=f"tq_{qg}_{pair}")
                for qt in range(4):
                    nc.tensor.transpose(quad[:, qt, :], zq[:, qt, :], ident)
                with nc.allow_low_precision(reason="zT copy"):
                    nc.vector.tensor_copy(
                        zT[:, pair, qg * 512 : (qg + 1) * 512],
                        quad[:].rearrange("p a b -> p (a b)"),
                    )

            def emit_op_chunk(qg, nh, ti, ysb, dma_split):
                tt = qg * 4 + ti
                ps = ps_b.tile([128, 512], f32, tag="b",
                               name=f"op_{qg}_{nh}_{ti}")
                for co in range(2):
                    nc.tensor.matmul(
                        ps[:],
                        zT[:, co, tt * 128 : (tt + 1) * 128],
                        wo[:, co, nh * 512 : (nh + 1) * 512],
                        start=(co == 0),
                        stop=(co == 1),
                    )
                nc.vector.tensor_copy(ysb[:, ti, :], ps[:])
                nper = 4 // dma_split
                if ti % nper == nper - 1:
                    t0 = tt - nper + 1
                    deng = nc.sync if (ti // nper + nh) % 2 == 0 else nc.scalar
                    deng.dma_start(
                        y_d[t0 * 128 : (tt + 1) * 128,
                            nh * 512 : (nh + 1) * 512].rearrange(
                            "(ti p) n -> p ti n", p=128
                        ),
                        ysb[:, ti - nper + 1 : ti + 1, :],
                    )

            def push_op_fillers(qg):
                dma_split = dma_splits[qg]
                for nh in range(2):
                    ysb = ypool.tile([128, 4, 512], f32, tag="y",
                                     name=f"ysb_{qg}_{nh}")
                    for ti in range(4):
                        fillers.append(
                            (("op", qg), 1024 * PE_NS,
                             lambda qg=qg, nh=nh, ti=ti, ysb=ysb,
                                    ds=dma_split:
                                 emit_op_chunk(qg, nh, ti, ysb, ds))
                        )

            def push_z_phase(qg, h, pts, kt2g, zq_box):
                """Queue head h's z chains + normalize (+ transpose) at the
                FRONT of the filler queue; they drain during head h+1's S
                phase (one-head software pipeline)."""
                box = {}

                def chain(qt):
                    if qt == 0:
                        box["zp"] = ps_z.tile([128, 4, 65], f32, tag="z",
                                              name=f"zp_{h}_{qg}")
                        if h % 2 == 0:
                            zq_box[h // 2] = zqpool.tile(
                                [128, 4, 128], bf16, tag="zq",
                                name=f"zq_{qg}_{h // 2}")
                    emit_zchain(h, qg, qt, pts, kt2g, box["zp"])

                def norm():
                    emit_norm(h, qg, box["zp"], zq_box[h // 2])
                    if h % 2 == 1:
                        emit_transpose(qg, h // 2, zq_box[h // 2])
                        if h == HLOC - 1:
                            # quarter finished: queue its out-proj (reads
                            # zT(qg), complete as of this point) and the
                            # next token group's QKV
                            push_op_fillers(qg)
                            if qg + 2 <= 3:
                                push_qkv_fillers(qg + 2)

                # the previous head's z thunks must fully precede this
                # head's (ps_z rotation + zq pair ordering)
                drain_class("z")
                thunks = []
                for qt in range(4):
                    ncols = (4 * qg + qt + 1) * 65
                    thunks.append(
                        (("z", qg, h), ncols * PE_NS,
                         lambda qt=qt: chain(qt))
                    )
                thunks.append((("z", qg, h), 0.0, norm))
                fillers[0:0] = thunks

            # ---- program ----
            emit_qkv(0)
            push_qkv_fillers(1)

            zq_box = {}
            for qg in range(4):
                groups = _groups(qg, sreg_w)
                kt2g = {}
                for gi, grp in enumerate(groups):
                    for kt, off, w in grp:
                        kt2g[kt] = (gi, off, w)
                if qg > 0:
                    # barrier: this quarter's S/z read QT/KT/VA of tg=qg
                    drain(("qkv", qg))
                for h in range(HLOC):
                    pts = []
                    for gi, grp in enumerate(groups):
                        pts.append(emit_sgrp(h, qg, gi, grp))
                        cum = grp[-1][1] + grp[-1][2]
                        exp_ns = cum * ACT_NS + 185.0
                        budget = (exp_ns * fill_scale + fill_pad
                                  - cum * PE_NS)
                        fill(max(0.0, budget))
                    push_z_phase(qg, h, pts, kt2g, zq_box)

            # drain the tail (queue can grow while draining)
            while fillers:
                _, _, thunk = _pop_next()
                thunk()

    nc.compile()
    return nc


def _pack_w(w):
    # [DM, C] -> [128, MO, C]: partition p holds rows {mo*128 + p}
    return np.ascontiguousarray(
        w.reshape(MO, 128, w.shape[1]).transpose(1, 0, 2)
    ).astype(ml_dtypes.bfloat16)


def make_in_maps(x, w_qkv, b_qkv, w_out):
    # multiplicative post-exp mask: 1 where k <= q (upper incl diag), else 0
    tri = np.tri(128, 128, 0, dtype=np.float32).T.astype(ml_dtypes.bfloat16)
    ident = np.eye(128, dtype=np.float32).astype(ml_dtypes.bfloat16)
    in_maps = []
    for core in range(8):
        b = core // 4
        hg = core % 4
        c0 = hg * CLOC
        csl = slice(c0, c0 + CLOC)

        # packed consts: [128, 264] bf16-typed raw columns
        cst = np.zeros((128, 264), np.uint16)
        cst[:, 0:128] = tri.view(np.uint16)
        cst[:, 128:256] = ident.view(np.uint16)
        bq = np.ascontiguousarray(
            b_qkv[csl].astype(np.float32).reshape(2, 128).T
        )
        bk = np.ascontiguousarray(
            b_qkv[DM + c0 : DM + c0 + CLOC].astype(np.float32).reshape(2, 128).T
        )
        cst[:, 256:260] = bq.view(np.uint16).reshape(128, 4)
        cst[:, 260:264] = bk.view(np.uint16).reshape(128, 4)

        wq_p = _pack_w(w_qkv[:, csl])
        wk_p = _pack_w(w_qkv[:, DM + c0 : DM + c0 + CLOC])
        # [128, ctj, MO, 128]: ctj = ct*2 + j (j=0 -> Q, j=1 -> K)
        wqk = np.stack(
            [wq_p[:, :, 0:128], wk_p[:, :, 0:128],
             wq_p[:, :, 128:256], wk_p[:, :, 128:256]],
            axis=1,
        )
        in_maps.append(
            {
                "xT": _pack_w(np.ascontiguousarray(x[b].T)),
                "wqk": np.ascontiguousarray(wqk),
                "wv": _pack_w(w_qkv[:, 2 * DM + c0 : 2 * DM + c0 + CLOC]),
                # wo: [CLOC, DM] -> [128, 2, DM]
                "wo": np.ascontiguousarray(
                    w_out[csl, :].reshape(2, 128, DM).transpose(1, 0, 2)
                ).astype(ml_dtypes.bfloat16),
                "cst": cst.view(ml_dtypes.bfloat16),
            }
        )
    return in_maps


def gather(results, b_qkv, w_out, b_out):
    # device skips the V bias; z_norm + b_v projects to a constant row:
    # y += b_v @ w_out, folded into the output bias here
    b_eff = (
        b_out.astype(np.float32)
        + b_qkv[2 * DM :].astype(np.float32) @ w_out.astype(np.float32)
    )
    out = np.empty((B, S, DM), np.float32)
    for b in range(B):
        acc = results[4 * b]["y"].astype(np.float32)
        for j in range(1, 4):
            acc = acc + results[4 * b + j]["y"]
        out[b] = acc + b_eff[None, :]
    return out


def kernel(x, w_qkv, b_qkv, w_out, b_out):
    x = np.asarray(x)
    w_qkv = np.asarray(w_qkv)
    b_qkv = np.asarray(b_qkv)
    w_out = np.asarray(w_out)
    b_out = np.asarray(b_out)

    if "nc" not in _CACHE:
        _CACHE["nc"] = build()
    nc = _CACHE["nc"]

    in_maps = make_in_maps(x, w_qkv, b_qkv, w_out)
    res = run_bass_kernel_spmd(nc, in_maps, core_ids=list(range(8)))
    return gather(res.results, b_qkv, w_out, b_out)


# revision 22
# speedup vs baseline: 1.0827x; 1.0425x over previous
"""Causal multi-head attention block (B=2, S=2048, D=1024, H=16) on 8 TRN2 cores.

Sharding: core i handles batch b = i//4 and head group hg = i%4 (4 heads =
256 model dims). Each core computes its heads' attention and a partial
output projection; the host sums the 4 partials per batch and adds b_out.

Per-core device pipeline (bf16 matmuls, fp32 PSUM accumulation):
  1. QKV. Q^T,K^T land as [head_cols, tokens] (lhsT = W, rhs = x^T);
     V lands as [tokens, head_cols] (lhsT = x^T tiles, rhs = W_v), stored
     augmented with a ones column so the z-matmul also produces softmax
     row sums.
  2. Attention per head, flash-style in the S^T = K.Q^T orientation over
     the causal lower triangle only: S^T[k_tile, q_span] -> exp on ScalarE
     (scale=1/8, no max subtraction; logits ~N(0,1)) -> P^T bf16 ->
     multiplicative 0/1 mask on diagonal blocks (GPSIMD) -> z[q_tile, 65]
     += P^T_chunk^T @ V_aug accumulated over k tiles in PSUM. The [q, d+1]
     z orientation makes each z matmul only 65 PE columns (vs a full
     q-span) and puts the softmax row sum in PSUM column 64 of the same
     partition as its query, so normalization is a per-partition
     tensor_scalar multiply fused into the PSUM->SBUF copy.
  3. z[q,d] tiles are transposed back to z^T[d,q] via PE transpose
     (identity matmul, 128 cols per 2-head tile) for the out-projection.
  4. Out-proj: y_partial[t, n] accumulated over the 256 local dims.

Program order is a fine-grained software pipeline: the attention loop is
a flat sequence over (q-quarter, head, k-group) with the z matmuls
lagging one group behind the S matmuls, and a filler queue (next token
group's QKV chains, previous quarters' out-proj chunks) drained between
S and z so the PE never waits on ScalarE exp. Host pre-packs all inputs
into SBUF layouts (bf16); the V bias is folded into the output bias on
the host (b_v @ w_out).
"""

import numpy as np
import ml_dtypes

import concourse.mybir as mybir
import concourse.tile as tile
from concourse import bacc
from concourse.bass_utils import run_bass_kernel_spmd

B = 2
S = 2048
DM = 1024
HD = 64
HLOC = 4                 # heads per core
CLOC = HLOC * HD         # local model dims (256)
MO = DM // 128           # 8 k-subtiles of the model dim
NKT = S // 128           # 16 key tiles

f32 = mybir.dt.float32
bf16 = mybir.dt.bfloat16
EXP = mybir.ActivationFunctionType.Exp

ACT_NS = 0.8333333333333334
PE_NS = 0.4166666666666667

_CACHE = {}


def _groups(qg, cap=1024):
    """Pack the causal k-tile spans of query quarter qg into exp groups of
    <= cap columns. Returns list of groups; each group is a list of
    (kt, offset_in_group, width)."""
    g0 = qg * 512
    last_kt = 4 * qg + 3
    groups, cur, cum = [], [], 0
    for kt in range(last_kt + 1):
        w = g0 + 512 - max(kt * 128, g0)
        if cum + w > cap:
            groups.append(cur)
            cur, cum = [], 0
        cur.append((kt, cum, w))
        cum += w
    groups.append(cur)
    return groups


def build(pt_bufs=16, zq_bufs=4, y_bufs=3, sreg_w=1024, fill_scale=1.0,
          fill_pad=150.0, dma_splits=(2, 2, 2, 4), tri_engine="gpsimd",
          z_delay=2):
    nc = bacc.Bacc("TRN2", target_bir_lowering=False, debug=False)

    xT_d = nc.dram_tensor("xT", [128, MO, S], bf16, kind="ExternalInput")
    # wqk grouped per QKV chain (ctj = ct*2+j) so each chain's weights are
    # one contiguous 2KB/partition DMA
    wqk_d = nc.dram_tensor("wqk", [128, 4, MO, 128], bf16, kind="ExternalInput")
    wv_d = nc.dram_tensor("wv", [128, MO, CLOC], bf16, kind="ExternalInput")
    wo_d = nc.dram_tensor("wo", [128, 2, DM], bf16, kind="ExternalInput")
    # consts packed as raw bf16 columns: tri[0:128], identity[128:256],
    # bq[256:260], bk[260:264] (f32 values bit-split across bf16 pairs)
    cst_d = nc.dram_tensor("cst", [128, 264], bf16, kind="ExternalInput")
    y_d = nc.dram_tensor("y", [S, DM], bf16, kind="ExternalOutput")

    with tile.TileContext(nc) as tc:
        with (
            tc.tile_pool(name="consts", bufs=1) as consts,
            tc.tile_pool(name="acts", bufs=1) as apool,
            tc.tile_pool(name="pt", bufs=pt_bufs) as ppool,
            tc.tile_pool(name="zq", bufs=zq_bufs) as zqpool,
            tc.tile_pool(name="norm", bufs=4) as spool,
            tc.tile_pool(name="ycopy", bufs=y_bufs) as ypool,
            # 8 PSUM banks: ps_s 2x[128,1024]=4 (QK logits), ps_z
            # 2x[128,4,65]=2 (z accumulators), ps_b 2x[128,512]=2
            # (QKV / V / out-proj chains and z transposes)
            tc.tile_pool(name="ps_s", bufs=2, space="PSUM") as ps_s,
            tc.tile_pool(name="ps_z", bufs=2, space="PSUM") as ps_z,
            tc.tile_pool(name="ps_b", bufs=2, space="PSUM") as ps_b,
        ):
            csb = consts.tile([128, 264], bf16)
            wqk = consts.tile([128, 4, MO, 128], bf16)
            wv = consts.tile([128, MO, CLOC], bf16)
            wo = consts.tile([128, 2, DM], bf16)
            xT = apool.tile([128, MO, S], bf16)

            # DMA order = consumption order (transfers serialize on the DMA
            # engines): first QKV chain's weights, then xT mo-pairs at the
            # pace the first chain consumes them, then the rest in bulk.
            nc.sync.dma_start(wqk[:, 0, :, :], wqk_d[:, 0, :, :])
            nc.scalar.dma_start(xT[:, 0:2, 0:512], xT_d[:, 0:2, 0:512])
            nc.sync.dma_start(csb[:], cst_d[:])
            nc.scalar.dma_start(xT[:, 2:4, 0:512], xT_d[:, 2:4, 0:512])
            nc.sync.dma_start(wqk[:, 1, :, :], wqk_d[:, 1, :, :])
            nc.scalar.dma_start(xT[:, 4:6, 0:512], xT_d[:, 4:6, 0:512])
            nc.sync.dma_start(wqk[:, 2:4, :, :], wqk_d[:, 2:4, :, :])
            nc.scalar.dma_start(xT[:, 6:8, 0:512], xT_d[:, 6:8, 0:512])
            nc.sync.dma_start(wv[:], wv_d[:])
            nc.scalar.dma_start(xT[:, :, 512:1024], xT_d[:, :, 512:1024])
            nc.sync.dma_start(xT[:, :, 1024:1536], xT_d[:, :, 1024:1536])
            nc.scalar.dma_start(xT[:, :, 1536:2048], xT_d[:, :, 1536:2048])
            nc.sync.dma_start(wo[:], wo_d[:])

            tri = csb[:, 0:128]
            ident = csb[:, 128:256]
            bq_sb = csb[:, 256:260].bitcast(f32)
            bk_sb = csb[:, 260:264].bitcast(f32)

            QT = apool.tile([128, 2, S], bf16)
            KT = apool.tile([128, 2, S], bf16)
            # V augmented: [t-part, kt, h, 0:64] = v dims, col 64 = ones
            VA = apool.tile([128, NKT, HLOC, 72], bf16)
            nc.vector.memset(VA[:, :, :, 64:65], 1.0)
            zT = apool.tile([128, 2, S], bf16)

            # ---- filler queue: PE work units drained while ScalarE exps ----
            fillers = []          # list of (key, pe_ns, thunk)
            fill_debt = [0.0]
            fill_count = [0]      # fill() invocations, for z-pop delay
            z_pushed_at = [0]

            def _pop_next():
                """z thunks once ScalarE has had time to produce their exp
                inputs (z_delay fill periods after push), then QKV (needed
                by the next quarter anyway), out-proj last (the only filler
                class available during the final ACT-bound quarter)."""
                z_ready = fill_count[0] - z_pushed_at[0] >= z_delay
                order = ("z", "qkv") if z_ready else ("qkv",)
                for cls in order:
                    for i, (k, ns, thunk) in enumerate(fillers):
                        if k[0] == cls:
                            return fillers.pop(i)
                for i, (k, ns, thunk) in enumerate(fillers):
                    if k[0] != "z":
                        return fillers.pop(i)
                return fillers.pop(0)

            def fill(budget_ns):
                fill_count[0] += 1
                budget = budget_ns + fill_debt[0]
                spent = 0.0
                while fillers and spent < budget:
                    _, ns, thunk = _pop_next()
                    thunk()
                    spent += ns
                fill_debt[0] = budget - spent if fillers else 0.0

            def drain(key):
                """Force-emit queued fillers matching key (dependency
                barrier: attention on quarter qg needs all of QKV(tg=qg))."""
                rest = []
                for k, ns, thunk in fillers:
                    if k == key:
                        thunk()
                    else:
                        rest.append((k, ns, thunk))
                fillers[:] = rest

            def drain_class(cls):
                rest = []
                for k, ns, thunk in fillers:
                    if k[0] == cls:
                        thunk()
                    else:
                        rest.append((k, ns, thunk))
                fillers[:] = rest

            def emit_qk_chain(tg, ct, j):
                tsl = slice(tg * 512, (tg + 1) * 512)
                dst, b_sb = ((QT, bq_sb), (KT, bk_sb))[j]
                ps = ps_b.tile([128, 512], f32, tag="b",
                               name=f"qk_{tg}_{ct}_{j}")
                for mo in range(MO):
                    nc.tensor.matmul(
                        ps[:],
                        wqk[:, ct * 2 + j, mo, :],
                        xT[:, mo, tsl],
                        start=(mo == 0),
                        stop=(mo == MO - 1),
                    )
                nc.vector.tensor_scalar_add(
                    dst[:, ct, tsl], ps[:], b_sb[:, ct : ct + 1]
                )

            def emit_v_chain(tg, ti):
                tt = tg * 4 + ti
                ps = ps_b.tile([128, 512], f32, tag="b", name=f"v_{tg}_{ti}")
                for mo in range(MO):
                    nc.tensor.matmul(
                        ps[:, 0:CLOC],
                        xT[:, mo, tt * 128 : (tt + 1) * 128],
                        wv[:, mo, :],
                        start=(mo == 0),
                        stop=(mo == MO - 1),
                    )
                nc.vector.tensor_copy(
                    VA[:, tt, :, 0:64],
                    ps[:, 0:CLOC].rearrange("p (h d) -> p h d", d=64),
                )

            def emit_qkv(tg):
                for ct in range(2):
                    for j in range(2):
                        emit_qk_chain(tg, ct, j)
                for ti in range(4):
                    emit_v_chain(tg, ti)

            def push_qkv_fillers(tg):
                for ct in range(2):
                    for j in range(2):
                        fillers.append(
                            (("qkv", tg), 4096 * PE_NS,
                             lambda tg=tg, ct=ct, j=j: emit_qk_chain(tg, ct, j))
                        )
                for ti in range(4):
                    fillers.append(
                        (("qkv", tg), 2048 * PE_NS,
                         lambda tg=tg, ti=ti: emit_v_chain(tg, ti))
                    )

            # ---- attention ----
            def emit_sgrp(h, qg, gi, grp):
                """S^T matmuls for one exp group + the exp + diag masks."""
                hp = (h % 2) * 64
                ct = h // 2
                g0 = qg * 512
                cum = grp[-1][1] + grp[-1][2]
                sreg = ps_s.tile([128, sreg_w], f32, tag="s",
                                 name=f"s_{h}_{qg}_{gi}")
                for kt, off, w in grp:
                    q0 = g0 + 512 - w
                    c0 = off
                    while c0 < off + w:
                        cw = min(off + w - c0, 512 - c0 % 512)
                        nc.tensor.matmul(
                            sreg[:, c0 : c0 + cw],
                            KT[hp : hp + 64, ct, kt * 128 : (kt + 1) * 128],
                            QT[hp : hp + 64, ct,
                               q0 + c0 - off : q0 + c0 - off + cw],
                        )
                        c0 += cw
                pT = ppool.tile([128, sreg_w], bf16, tag="pT",
                                name=f"pT_{h}_{qg}_{gi}")
                nc.scalar.activation(pT[:, :cum], sreg[:, :cum], EXP,
                                     scale=0.125)
                teng = nc.gpsimd if tri_engine == "gpsimd" else nc.vector
                for kt, off, w in grp:
                    if kt * 128 >= g0:  # diagonal block leads the span
                        teng.tensor_mul(
                            pT[:, off : off + 128],
                            pT[:, off : off + 128],
                            tri[:],
                        )
                return pT

            def emit_zchain(h, qg, qt, pts, kt2g, zp):
                """z[q,65] = sum_kt pT_chunk^T @ V_aug: one sequential PSUM
                accumulation chain per q-tile (a PSUM bank supports only one
                open accumulation group at a time)."""
                g0 = qg * 512
                qa = 4 * qg + qt
                for kt in range(qa + 1):
                    gi, off, w = kt2g[kt]
                    q0 = g0 + 512 - w
                    c0 = off + (g0 + qt * 128) - q0
                    nc.tensor.matmul(
                        zp[:, qt, 0:65],
                        pts[gi][:, c0 : c0 + 128],
                        VA[:, kt, h, 0:65],
                        start=(kt == 0),
                        stop=(kt == qa),
                    )

            def emit_norm(h, qg, zp, zq):
                """1/rowsum fused into the PSUM->SBUF copy of z."""
                hp = (h % 2) * 64
                rec = spool.tile([128, 4, 1], f32, tag="rec",
                                 name=f"rec_{h}_{qg}")
                nc.vector.reciprocal(rec[:], zp[:, :, 64:65])
                with nc.allow_low_precision(reason="attn out to bf16"):
                    for qt in range(4):
                        nc.vector.tensor_scalar_mul(
                            zq[:, qt, hp : hp + 64],
                            zp[:, qt, 0:64],
                            rec[:, qt, :],
                        )

            def emit_transpose(qg, pair, zq):
                """zq [q,128d] -> zT [128d, q] via PE transpose of 4 tiles."""
                quad = ps_b.tile([128, 4, 128], bf16, tag="b",
                                 name=f"tq_{qg}_{pair}")
                for qt in range(4):
                    nc.tensor.transpose(quad[:, qt, :], zq[:, qt, :], ident)
                with nc.allow_low_precision(reason="zT copy"):
                    nc.vector.tensor_copy(
                        zT[:, pair, qg * 512 : (qg + 1) * 512],
                        quad[:].rearrange("p a b -> p (a b)"),
                    )

            def emit_op_chunk(qg, nh, ti, ysb, dma_split):
                tt = qg * 4 + ti
                ps = ps_b.tile([128, 512], f32, tag="b",
                               name=f"op_{qg}_{nh}_{ti}")
                for co in range(2):
                    nc.tensor.matmul(
                        ps[:],
                        zT[:, co, tt * 128 : (tt + 1) * 128],
                        wo[:, co, nh * 512 : (nh + 1) * 512],
                        start=(co == 0),
                        stop=(co == 1),
                    )
                with nc.allow_low_precision(reason="y partial to bf16"):
                    nc.vector.tensor_copy(ysb[:, ti, :], ps[:])
                nper = 4 // dma_split
                if ti % nper == nper - 1:
                    t0 = tt - nper + 1
                    deng = nc.sync if (ti // nper + nh) % 2 == 0 else nc.scalar
                    deng.dma_start(
                        y_d[t0 * 128 : (tt + 1) * 128,
                            nh * 512 : (nh + 1) * 512].rearrange(
                            "(ti p) n -> p ti n", p=128
                        ),
                        ysb[:, ti - nper + 1 : ti + 1, :],
                    )

            def push_op_fillers(qg):
                dma_split = dma_splits[qg]
                for nh in range(2):
                    ysb = ypool.tile([128, 4, 512], bf16, tag="y",
                                     name=f"ysb_{qg}_{nh}")
                    for ti in range(4):
                        fillers.append(
                            (("op", qg), 1024 * PE_NS,
                             lambda qg=qg, nh=nh, ti=ti, ysb=ysb,
                                    ds=dma_split:
                                 emit_op_chunk(qg, nh, ti, ysb, ds))
                        )

            def push_z_phase(qg, h, pts, kt2g, zq_box):
                """Queue head h's z chains + normalize (+ transpose) at the
                FRONT of the filler queue; they drain during head h+1's S
                phase (one-head software pipeline)."""
                box = {}

                def chain(qt):
                    if qt == 0:
                        box["zp"] = ps_z.tile([128, 4, 65], f32, tag="z",
                                              name=f"zp_{h}_{qg}")
                        if h % 2 == 0:
                            zq_box[h // 2] = zqpool.tile(
                                [128, 4, 128], bf16, tag="zq",
                                name=f"zq_{qg}_{h // 2}")
                    emit_zchain(h, qg, qt, pts, kt2g, box["zp"])

                def norm():
                    emit_norm(h, qg, box["zp"], zq_box[h // 2])
                    if h % 2 == 1:
                        emit_transpose(qg, h // 2, zq_box[h // 2])
                        if h == HLOC - 1:
                            # quarter finished: queue its out-proj (reads
                            # zT(qg), complete as of this point) and the
                            # next token group's QKV
                            push_op_fillers(qg)
                            if qg + 2 <= 3:
                                push_qkv_fillers(qg + 2)

                # the previous head's z thunks must fully precede this
                # head's (ps_z rotation + zq pair ordering)
                drain_class("z")
                thunks = []
                for qt in range(4):
                    ncols = (4 * qg + qt + 1) * 65
                    thunks.append(
                        (("z", qg, h), ncols * PE_NS,
                         lambda qt=qt: chain(qt))
                    )
                thunks.append((("z", qg, h), 0.0, norm))
                fillers[0:0] = thunks
                z_pushed_at[0] = fill_count[0]

            # ---- program ----
            emit_qkv(0)
            push_qkv_fillers(1)

            zq_box = {}
            for qg in range(4):
                groups = _groups(qg, sreg_w)
                kt2g = {}
                for gi, grp in enumerate(groups):
                    for kt, off, w in grp:
                        kt2g[kt] = (gi, off, w)
                if qg > 0:
                    # barrier: this quarter's S/z read QT/KT/VA of tg=qg
                    drain(("qkv", qg))
                for h in range(HLOC):
                    pts = []
                    for gi, grp in enumerate(groups):
                        pts.append(emit_sgrp(h, qg, gi, grp))
                        cum = grp[-1][1] + grp[-1][2]
                        exp_ns = cum * ACT_NS + 185.0
                        budget = (exp_ns * fill_scale + fill_pad
                                  - cum * PE_NS)
                        fill(max(0.0, budget))
                    push_z_phase(qg, h, pts, kt2g, zq_box)

            # drain the tail (queue can grow while draining)
            while fillers:
                _, _, thunk = _pop_next()
                thunk()

    nc.compile()
    return nc


def _pack_w(w):
    # [DM, C] -> [128, MO, C]: partition p holds rows {mo*128 + p}
    return np.ascontiguousarray(
        w.reshape(MO, 128, w.shape[1]).transpose(1, 0, 2)
    ).astype(ml_dtypes.bfloat16)


def make_in_maps(x, w_qkv, b_qkv, w_out):
    # multiplicative post-exp mask: 1 where k <= q (upper incl diag), else 0
    tri = np.tri(128, 128, 0, dtype=np.float32).T.astype(ml_dtypes.bfloat16)
    ident = np.eye(128, dtype=np.float32).astype(ml_dtypes.bfloat16)
    in_maps = []
    for core in range(8):
        b = core // 4
        hg = core % 4
        c0 = hg * CLOC
        csl = slice(c0, c0 + CLOC)

        # packed consts: [128, 264] bf16-typed raw columns
        cst = np.zeros((128, 264), np.uint16)
        cst[:, 0:128] = tri.view(np.uint16)
        cst[:, 128:256] = ident.view(np.uint16)
        bq = np.ascontiguousarray(
            b_qkv[csl].astype(np.float32).reshape(2, 128).T
        )
        bk = np.ascontiguousarray(
            b_qkv[DM + c0 : DM + c0 + CLOC].astype(np.float32).reshape(2, 128).T
        )
        cst[:, 256:260] = bq.view(np.uint16).reshape(128, 4)
        cst[:, 260:264] = bk.view(np.uint16).reshape(128, 4)

        wq_p = _pack_w(w_qkv[:, csl])
        wk_p = _pack_w(w_qkv[:, DM + c0 : DM + c0 + CLOC])
        # [128, ctj, MO, 128]: ctj = ct*2 + j (j=0 -> Q, j=1 -> K)
        wqk = np.stack(
            [wq_p[:, :, 0:128], wk_p[:, :, 0:128],
             wq_p[:, :, 128:256], wk_p[:, :, 128:256]],
            axis=1,
        )
        in_maps.append(
            {
                "xT": _pack_w(np.ascontiguousarray(x[b].T)),
                "wqk": np.ascontiguousarray(wqk),
                "wv": _pack_w(w_qkv[:, 2 * DM + c0 : 2 * DM + c0 + CLOC]),
                # wo: [CLOC, DM] -> [128, 2, DM]
                "wo": np.ascontiguousarray(
                    w_out[csl, :].reshape(2, 128, DM).transpose(1, 0, 2)
                ).astype(ml_dtypes.bfloat16),
                "cst": cst.view(ml_dtypes.bfloat16),
            }
        )
    return in_maps


def gather(results, b_qkv, w_out, b_out):
    # device skips the V bias; z_norm + b_v projects to a constant row:
    # y += b_v @ w_out, folded into the output bias here
    b_eff = (
        b_out.astype(np.float32)
        + b_qkv[2 * DM :].astype(np.float32) @ w_out.astype(np.float32)
    )
    out = np.empty((B, S, DM), np.float32)
    for b in range(B):
        acc = results[4 * b]["y"].astype(np.float32)
        for j in range(1, 4):
            acc = acc + results[4 * b + j]["y"]
        out[b] = acc + b_eff[None, :]
    return out


def kernel(x, w_qkv, b_qkv, w_out, b_out):
    x = np.asarray(x)
    w_qkv = np.asarray(w_qkv)
    b_qkv = np.asarray(b_qkv)
    w_out = np.asarray(w_out)
    b_out = np.asarray(b_out)

    if "nc" not in _CACHE:
        _CACHE["nc"] = build()
    nc = _CACHE["nc"]

    in_maps = make_in_maps(x, w_qkv, b_qkv, w_out)
    res = run_bass_kernel_spmd(nc, in_maps, core_ids=list(range(8)))
    return gather(res.results, b_qkv, w_out, b_out)


# revision 23
# speedup vs baseline: 1.0929x; 1.0095x over previous
"""Causal multi-head attention block (B=2, S=2048, D=1024, H=16) on 8 TRN2 cores.

Sharding: core i handles batch b = i//4 and head group hg = i%4 (4 heads =
256 model dims). Each core computes its heads' attention and a partial
output projection; the host sums the 4 partials per batch and adds b_out.

Per-core device pipeline (bf16 matmuls, fp32 PSUM accumulation):
  1. QKV. Q^T,K^T land as [head_cols, tokens] (lhsT = W, rhs = x^T);
     V lands as [tokens, head_cols] (lhsT = x^T tiles, rhs = W_v), stored
     augmented with a ones column so the z-matmul also produces softmax
     row sums.
  2. Attention per head, flash-style in the S^T = K.Q^T orientation over
     the causal lower triangle only: S^T[k_tile, q_span] -> exp on ScalarE
     (scale=1/8, no max subtraction; logits ~N(0,1)) -> P^T bf16 ->
     multiplicative 0/1 mask on diagonal blocks (GPSIMD) -> z[q_tile, 65]
     += P^T_chunk^T @ V_aug accumulated over k tiles in PSUM. The [q, d+1]
     z orientation makes each z matmul only 65 PE columns (vs a full
     q-span) and puts the softmax row sum in PSUM column 64 of the same
     partition as its query, so normalization is a per-partition
     tensor_scalar multiply fused into the PSUM->SBUF copy.
  3. z[q,d] tiles are transposed back to z^T[d,q] via PE transpose
     (identity matmul, 128 cols per 2-head tile) for the out-projection.
  4. Out-proj: y_partial[t, n] accumulated over the 256 local dims.

Program order is a fine-grained software pipeline: the attention loop is
a flat sequence over (q-quarter, head, k-group) with the z matmuls
lagging one group behind the S matmuls, and a filler queue (next token
group's QKV chains, previous quarters' out-proj chunks) drained between
S and z so the PE never waits on ScalarE exp. Host pre-packs all inputs
into SBUF layouts (bf16); the V bias is folded into the output bias on
the host (b_v @ w_out).
"""

import numpy as np
import ml_dtypes

import concourse.mybir as mybir
import concourse.tile as tile
from concourse import bacc
from concourse.bass_utils import run_bass_kernel_spmd

B = 2
S = 2048
DM = 1024
HD = 64
HLOC = 4                 # heads per core
CLOC = HLOC * HD         # local model dims (256)
MO = DM // 128           # 8 k-subtiles of the model dim
NKT = S // 128           # 16 key tiles

f32 = mybir.dt.float32
bf16 = mybir.dt.bfloat16
EXP = mybir.ActivationFunctionType.Exp

ACT_NS = 0.8333333333333334
PE_NS = 0.4166666666666667

_CACHE = {}


def _groups(qg, cap=1024):
    """Pack the causal k-tile spans of query quarter qg into exp groups of
    <= cap columns. Returns list of groups; each group is a list of
    (kt, offset_in_group, width)."""
    g0 = qg * 512
    last_kt = 4 * qg + 3
    groups, cur, cum = [], [], 0
    for kt in range(last_kt + 1):
        w = g0 + 512 - max(kt * 128, g0)
        if cum + w > cap:
            groups.append(cur)
            cur, cum = [], 0
        cur.append((kt, cum, w))
        cum += w
    groups.append(cur)
    return groups


def build(pt_bufs=16, zq_bufs=4, y_bufs=3, sreg_w=1024, fill_scale=1.0,
          fill_pad=50.0, dma_splits=(2, 2, 2, 4), tri_engine="gpsimd",
          z_delay=2):
    nc = bacc.Bacc("TRN2", target_bir_lowering=False, debug=False)

    xT_d = nc.dram_tensor("xT", [128, MO, S], bf16, kind="ExternalInput")
    # wqk grouped per QKV chain (ctj = ct*2+j) so each chain's weights are
    # one contiguous 2KB/partition DMA
    wqk_d = nc.dram_tensor("wqk", [128, 4, MO, 128], bf16, kind="ExternalInput")
    wv_d = nc.dram_tensor("wv", [128, MO, CLOC], bf16, kind="ExternalInput")
    wo_d = nc.dram_tensor("wo", [128, 2, DM], bf16, kind="ExternalInput")
    # consts packed as raw bf16 columns: tri[0:128], identity[128:256],
    # bq[256:260], bk[260:264] (f32 values bit-split across bf16 pairs)
    cst_d = nc.dram_tensor("cst", [128, 264], bf16, kind="ExternalInput")
    y_d = nc.dram_tensor("y", [S, DM], bf16, kind="ExternalOutput")

    with tile.TileContext(nc) as tc:
        with (
            tc.tile_pool(name="consts", bufs=1) as consts,
            tc.tile_pool(name="acts", bufs=1) as apool,
            tc.tile_pool(name="pt", bufs=pt_bufs) as ppool,
            tc.tile_pool(name="zq", bufs=zq_bufs) as zqpool,
            tc.tile_pool(name="norm", bufs=4) as spool,
            tc.tile_pool(name="ycopy", bufs=y_bufs) as ypool,
            # 8 PSUM banks: ps_s 2x[128,1024]=4 (QK logits), ps_z
            # 2x[128,4,65]=2 (z accumulators), ps_b 2x[128,512]=2
            # (QKV / V / out-proj chains and z transposes)
            tc.tile_pool(name="ps_s", bufs=2, space="PSUM") as ps_s,
            tc.tile_pool(name="ps_z", bufs=2, space="PSUM") as ps_z,
            tc.tile_pool(name="ps_b", bufs=2, space="PSUM") as ps_b,
        ):
            csb = consts.tile([128, 264], bf16)
            wqk = consts.tile([128, 4, MO, 128], bf16)
            wv = consts.tile([128, MO, CLOC], bf16)
            wo = consts.tile([128, 2, DM], bf16)
            xT = apool.tile([128, MO, S], bf16)

            # DMA order = consumption order (transfers serialize on the DMA
            # engines): first QKV chain's weights, then xT mo-pairs at the
            # pace the first chain consumes them, then the rest in bulk.
            nc.sync.dma_start(wqk[:, 0, 0:4, :], wqk_d[:, 0, 0:4, :])
            nc.scalar.dma_start(xT[:, 0:2, 0:512], xT_d[:, 0:2, 0:512])
            nc.sync.dma_start(wqk[:, 0, 4:MO, :], wqk_d[:, 0, 4:MO, :])
            nc.sync.dma_start(csb[:], cst_d[:])
            nc.scalar.dma_start(xT[:, 2:4, 0:512], xT_d[:, 2:4, 0:512])
            nc.sync.dma_start(wqk[:, 1, :, :], wqk_d[:, 1, :, :])
            nc.scalar.dma_start(xT[:, 4:6, 0:512], xT_d[:, 4:6, 0:512])
            nc.sync.dma_start(wqk[:, 2:4, :, :], wqk_d[:, 2:4, :, :])
            nc.scalar.dma_start(xT[:, 6:8, 0:512], xT_d[:, 6:8, 0:512])
            nc.sync.dma_start(wv[:], wv_d[:])
            nc.scalar.dma_start(xT[:, :, 512:1024], xT_d[:, :, 512:1024])
            nc.sync.dma_start(xT[:, :, 1024:1536], xT_d[:, :, 1024:1536])
            nc.scalar.dma_start(xT[:, :, 1536:2048], xT_d[:, :, 1536:2048])
            nc.sync.dma_start(wo[:], wo_d[:])

            tri = csb[:, 0:128]
            ident = csb[:, 128:256]
            bq_sb = csb[:, 256:260].bitcast(f32)
            bk_sb = csb[:, 260:264].bitcast(f32)

            QT = apool.tile([128, 2, S], bf16)
            KT = apool.tile([128, 2, S], bf16)
            # V augmented: [t-part, kt, h, 0:64] = v dims, col 64 = ones
            VA = apool.tile([128, NKT, HLOC, 72], bf16)
            nc.vector.memset(VA[:, :, :, 64:65], 1.0)
            zT = apool.tile([128, 2, S], bf16)

            # ---- filler queue: PE work units drained while ScalarE exps ----
            fillers = []          # list of (key, pe_ns, thunk)
            fill_debt = [0.0]
            fill_count = [0]      # fill() invocations, for z-pop delay
            z_pushed_at = [0]

            def _pop_next():
                """z thunks once ScalarE has had time to produce their exp
                inputs (z_delay fill periods after push), then QKV (needed
                by the next quarter anyway), out-proj last (the only filler
                class available during the final ACT-bound quarter)."""
                z_ready = fill_count[0] - z_pushed_at[0] >= z_delay
                order = ("z", "qkv") if z_ready else ("qkv",)
                for cls in order:
                    for i, (k, ns, thunk) in enumerate(fillers):
                        if k[0] == cls:
                            return fillers.pop(i)
                for i, (k, ns, thunk) in enumerate(fillers):
                    if k[0] != "z":
                        return fillers.pop(i)
                return fillers.pop(0)

            def fill(budget_ns):
                fill_count[0] += 1
                budget = budget_ns + fill_debt[0]
                spent = 0.0
                while fillers and spent < budget:
                    _, ns, thunk = _pop_next()
                    thunk()
                    spent += ns
                fill_debt[0] = budget - spent if fillers else 0.0

            def drain(key):
                """Force-emit queued fillers matching key (dependency
                barrier: attention on quarter qg needs all of QKV(tg=qg))."""
                rest = []
                for k, ns, thunk in fillers:
                    if k == key:
                        thunk()
                    else:
                        rest.append((k, ns, thunk))
                fillers[:] = rest

            def drain_class(cls):
                rest = []
                for k, ns, thunk in fillers:
                    if k[0] == cls:
                        thunk()
                    else:
                        rest.append((k, ns, thunk))
                fillers[:] = rest

            def emit_qk_chain(tg, ct, j):
                tsl = slice(tg * 512, (tg + 1) * 512)
                dst, b_sb = ((QT, bq_sb), (KT, bk_sb))[j]
                ps = ps_b.tile([128, 512], f32, tag="b",
                               name=f"qk_{tg}_{ct}_{j}")
                for mo in range(MO):
                    nc.tensor.matmul(
                        ps[:],
                        wqk[:, ct * 2 + j, mo, :],
                        xT[:, mo, tsl],
                        start=(mo == 0),
                        stop=(mo == MO - 1),
                    )
                nc.vector.tensor_scalar_add(
                    dst[:, ct, tsl], ps[:], b_sb[:, ct : ct + 1]
                )

            def emit_v_chain(tg, ti):
                tt = tg * 4 + ti
                ps = ps_b.tile([128, 512], f32, tag="b", name=f"v_{tg}_{ti}")
                for mo in range(MO):
                    nc.tensor.matmul(
                        ps[:, 0:CLOC],
                        xT[:, mo, tt * 128 : (tt + 1) * 128],
                        wv[:, mo, :],
                        start=(mo == 0),
                        stop=(mo == MO - 1),
                    )
                nc.vector.tensor_copy(
                    VA[:, tt, :, 0:64],
                    ps[:, 0:CLOC].rearrange("p (h d) -> p h d", d=64),
                )

            def emit_qkv(tg):
                for ct in range(2):
                    for j in range(2):
                        emit_qk_chain(tg, ct, j)
                for ti in range(4):
                    emit_v_chain(tg, ti)

            def push_qkv_fillers(tg):
                for ct in range(2):
                    for j in range(2):
                        fillers.append(
                            (("qkv", tg), 4096 * PE_NS,
                             lambda tg=tg, ct=ct, j=j: emit_qk_chain(tg, ct, j))
                        )
                for ti in range(4):
                    fillers.append(
                        (("qkv", tg), 2048 * PE_NS,
                         lambda tg=tg, ti=ti: emit_v_chain(tg, ti))
                    )

            # ---- attention ----
            def emit_sgrp(h, qg, gi, grp):
                """S^T matmuls for one exp group + the exp + diag masks."""
                hp = (h % 2) * 64
                ct = h // 2
                g0 = qg * 512
                cum = grp[-1][1] + grp[-1][2]
                sreg = ps_s.tile([128, sreg_w], f32, tag="s",
                                 name=f"s_{h}_{qg}_{gi}")
                for kt, off, w in grp:
                    q0 = g0 + 512 - w
                    c0 = off
                    while c0 < off + w:
                        cw = min(off + w - c0, 512 - c0 % 512)
                        nc.tensor.matmul(
                            sreg[:, c0 : c0 + cw],
                            KT[hp : hp + 64, ct, kt * 128 : (kt + 1) * 128],
                            QT[hp : hp + 64, ct,
                               q0 + c0 - off : q0 + c0 - off + cw],
                        )
                        c0 += cw
                pT = ppool.tile([128, sreg_w], bf16, tag="pT",
                                name=f"pT_{h}_{qg}_{gi}")
                nc.scalar.activation(pT[:, :cum], sreg[:, :cum], EXP,
                                     scale=0.125)
                teng = nc.gpsimd if tri_engine == "gpsimd" else nc.vector
                for kt, off, w in grp:
                    if kt * 128 >= g0:  # diagonal block leads the span
                        teng.tensor_mul(
                            pT[:, off : off + 128],
                            pT[:, off : off + 128],
                            tri[:],
                        )
                return pT

            def emit_zchain(h, qg, qt, pts, kt2g, zp):
                """z[q,65] = sum_kt pT_chunk^T @ V_aug: one sequential PSUM
                accumulation chain per q-tile (a PSUM bank supports only one
                open accumulation group at a time)."""
                g0 = qg * 512
                qa = 4 * qg + qt
                for kt in range(qa + 1):
                    gi, off, w = kt2g[kt]
                    q0 = g0 + 512 - w
                    c0 = off + (g0 + qt * 128) - q0
                    nc.tensor.matmul(
                        zp[:, qt, 0:65],
                        pts[gi][:, c0 : c0 + 128],
                        VA[:, kt, h, 0:65],
                        start=(kt == 0),
                        stop=(kt == qa),
                    )

            def emit_norm(h, qg, zp, zq):
                """1/rowsum fused into the PSUM->SBUF copy of z."""
                hp = (h % 2) * 64
                rec = spool.tile([128, 4, 1], f32, tag="rec",
                                 name=f"rec_{h}_{qg}")
                nc.vector.reciprocal(rec[:], zp[:, :, 64:65])
                with nc.allow_low_precision(reason="attn out to bf16"):
                    for qt in range(4):
                        nc.vector.tensor_scalar_mul(
                            zq[:, qt, hp : hp + 64],
                            zp[:, qt, 0:64],
                            rec[:, qt, :],
                        )

            def emit_transpose(qg, pair, zq):
                """zq [q,128d] -> zT [128d, q] via PE transpose of 4 tiles."""
                quad = ps_b.tile([128, 4, 128], bf16, tag="b",
                                 name=f"tq_{qg}_{pair}")
                for qt in range(4):
                    nc.tensor.transpose(quad[:, qt, :], zq[:, qt, :], ident)
                with nc.allow_low_precision(reason="zT copy"):
                    nc.vector.tensor_copy(
                        zT[:, pair, qg * 512 : (qg + 1) * 512],
                        quad[:].rearrange("p a b -> p (a b)"),
                    )

            def emit_op_chunk(qg, nh, ti, ysb, dma_split):
                tt = qg * 4 + ti
                ps = ps_b.tile([128, 512], f32, tag="b",
                               name=f"op_{qg}_{nh}_{ti}")
                for co in range(2):
                    nc.tensor.matmul(
                        ps[:],
                        zT[:, co, tt * 128 : (tt + 1) * 128],
                        wo[:, co, nh * 512 : (nh + 1) * 512],
                        start=(co == 0),
                        stop=(co == 1),
                    )
                with nc.allow_low_precision(reason="y partial to bf16"):
                    if qg == 3:
                        nc.scalar.copy(ysb[:, ti, :], ps[:])
                    else:
                        nc.vector.tensor_copy(ysb[:, ti, :], ps[:])
                nper = 4 // dma_split
                if ti % nper == nper - 1:
                    t0 = tt - nper + 1
                    deng = nc.sync if (ti // nper + nh) % 2 == 0 else nc.scalar
                    deng.dma_start(
                        y_d[t0 * 128 : (tt + 1) * 128,
                            nh * 512 : (nh + 1) * 512].rearrange(
                            "(ti p) n -> p ti n", p=128
                        ),
                        ysb[:, ti - nper + 1 : ti + 1, :],
                    )

            def push_op_fillers(qg):
                dma_split = dma_splits[qg]
                for nh in range(2):
                    ysb = ypool.tile([128, 4, 512], bf16, tag="y",
                                     name=f"ysb_{qg}_{nh}")
                    for ti in range(4):
                        fillers.append(
                            (("op", qg), 1024 * PE_NS,
                             lambda qg=qg, nh=nh, ti=ti, ysb=ysb,
                                    ds=dma_split:
                                 emit_op_chunk(qg, nh, ti, ysb, ds))
                        )

            def push_z_phase(qg, h, pts, kt2g, zq_box):
                """Queue head h's z chains + normalize (+ transpose) at the
                FRONT of the filler queue; they drain during head h+1's S
                phase (one-head software pipeline)."""
                box = {}

                def chain(qt):
                    if qt == 0:
                        box["zp"] = ps_z.tile([128, 4, 65], f32, tag="z",
                                              name=f"zp_{h}_{qg}")
                        if h % 2 == 0:
                            zq_box[h // 2] = zqpool.tile(
                                [128, 4, 128], bf16, tag="zq",
                                name=f"zq_{qg}_{h // 2}")
                    emit_zchain(h, qg, qt, pts, kt2g, box["zp"])

                def norm():
                    emit_norm(h, qg, box["zp"], zq_box[h // 2])
                    if h % 2 == 1:
                        emit_transpose(qg, h // 2, zq_box[h // 2])
                        if h == HLOC - 1:
                            # quarter finished: queue its out-proj (reads
                            # zT(qg), complete as of this point) and the
                            # next token group's QKV
                            push_op_fillers(qg)
                            if qg + 2 <= 3:
                                push_qkv_fillers(qg + 2)

                # the previous head's z thunks must fully precede this
                # head's (ps_z rotation + zq pair ordering)
                drain_class("z")
                thunks = []
                for qt in range(4):
                    ncols = (4 * qg + qt + 1) * 65
                    thunks.append(
                        (("z", qg, h), ncols * PE_NS,
                         lambda qt=qt: chain(qt))
                    )
                thunks.append((("z", qg, h), 0.0, norm))
                fillers[0:0] = thunks
                z_pushed_at[0] = fill_count[0]

            # ---- program ----
            emit_qkv(0)
            push_qkv_fillers(1)

            zq_box = {}
            for qg in range(4):
                groups = _groups(qg, sreg_w)
                kt2g = {}
                for gi, grp in enumerate(groups):
                    for kt, off, w in grp:
                        kt2g[kt] = (gi, off, w)
                if qg > 0:
                    # barrier: this quarter's S/z read QT/KT/VA of tg=qg
                    drain(("qkv", qg))
                for h in range(HLOC):
                    pts = []
                    for gi, grp in enumerate(groups):
                        pts.append(emit_sgrp(h, qg, gi, grp))
                        cum = grp[-1][1] + grp[-1][2]
                        exp_ns = cum * ACT_NS + 185.0
                        budget = (exp_ns * fill_scale + fill_pad
                                  - cum * PE_NS)
                        fill(max(0.0, budget))
                    push_z_phase(qg, h, pts, kt2g, zq_box)

            # drain the tail (queue can grow while draining)
            while fillers:
                _, _, thunk = _pop_next()
                thunk()

    nc.compile()
    return nc


def _pack_w(w):
    # [DM, C] -> [128, MO, C]: partition p holds rows {mo*128 + p}
    return np.ascontiguousarray(
        w.reshape(MO, 128, w.shape[1]).transpose(1, 0, 2)
    ).astype(ml_dtypes.bfloat16)


def make_in_maps(x, w_qkv, b_qkv, w_out):
    # multiplicative post-exp mask: 1 where k <= q (upper incl diag), else 0
    tri = np.tri(128, 128, 0, dtype=np.float32).T.astype(ml_dtypes.bfloat16)
    ident = np.eye(128, dtype=np.float32).astype(ml_dtypes.bfloat16)
    in_maps = []
    for core in range(8):
        b = core // 4
        hg = core % 4
        c0 = hg * CLOC
        csl = slice(c0, c0 + CLOC)

        # packed consts: [128, 264] bf16-typed raw columns
        cst = np.zeros((128, 264), np.uint16)
        cst[:, 0:128] = tri.view(np.uint16)
        cst[:, 128:256] = ident.view(np.uint16)
        bq = np.ascontiguousarray(
            b_qkv[csl].astype(np.float32).reshape(2, 128).T
        )
        bk = np.ascontiguousarray(
            b_qkv[DM + c0 : DM + c0 + CLOC].astype(np.float32).reshape(2, 128).T
        )
        cst[:, 256:260] = bq.view(np.uint16).reshape(128, 4)
        cst[:, 260:264] = bk.view(np.uint16).reshape(128, 4)

        wq_p = _pack_w(w_qkv[:, csl])
        wk_p = _pack_w(w_qkv[:, DM + c0 : DM + c0 + CLOC])
        # [128, ctj, MO, 128]: ctj = ct*2 + j (j=0 -> Q, j=1 -> K)
        wqk = np.stack(
            [wq_p[:, :, 0:128], wk_p[:, :, 0:128],
             wq_p[:, :, 128:256], wk_p[:, :, 128:256]],
            axis=1,
        )
        in_maps.append(
            {
                "xT": _pack_w(np.ascontiguousarray(x[b].T)),
                "wqk": np.ascontiguousarray(wqk),
                "wv": _pack_w(w_qkv[:, 2 * DM + c0 : 2 * DM + c0 + CLOC]),
                # wo: [CLOC, DM] -> [128, 2, DM]
                "wo": np.ascontiguousarray(
                    w_out[csl, :].reshape(2, 128, DM).transpose(1, 0, 2)
                ).astype(ml_dtypes.bfloat16),
                "cst": cst.view(ml_dtypes.bfloat16),
            }
        )
    return in_maps


def gather(results, b_qkv, w_out, b_out):
    # device skips the V bias; z_norm + b_v projects to a constant row:
    # y += b_v @ w_out, folded into the output bias here
    b_eff = (
        b_out.astype(np.float32)
        + b_qkv[2 * DM :].astype(np.float32) @ w_out.astype(np.float32)
    )
    out = np.empty((B, S, DM), np.float32)
    for b in range(B):
        acc = results[4 * b]["y"].astype(np.float32)
        for j in range(1, 4):
            acc = acc + results[4 * b + j]["y"]
        out[b] = acc + b_eff[None, :]
    return out


def kernel(x, w_qkv, b_qkv, w_out, b_out):
    x = np.asarray(x)
    w_qkv = np.asarray(w_qkv)
    b_qkv = np.asarray(b_qkv)
    w_out = np.asarray(w_out)
    b_out = np.asarray(b_out)

    if "nc" not in _CACHE:
        _CACHE["nc"] = build()
    nc = _CACHE["nc"]

    in_maps = make_in_maps(x, w_qkv, b_qkv, w_out)
    res = run_bass_kernel_spmd(nc, in_maps, core_ids=list(range(8)))
    return gather(res.results, b_qkv, w_out, b_out)
